# revision 1
# baseline (speedup 1.0000x reference)
"""Trainium2 Bass kernel for nn_ConsciousnessMonitor (histogram_binning).

kernel(**inputs) takes FULL unsharded numpy inputs, returns the full (9,)
float32 output. Shards state_history along time across 8 NeuronCores:
masked means via PE matmul while streaming, min/max + joint-histogram MI
with two small AllReduces, differentiation branch replicated per core.

Self-contained: shapes/sharding hardcoded; reads no sibling files.
"""
import numpy as np

import concourse.bacc as bacc
import concourse.tile as tile
import concourse.mybir as mybir
from concourse.bass_utils import run_bass_kernel_spmd
from concourse.masks import make_identity

F32 = mybir.dt.float32
I32 = mybir.dt.int32
AX = mybir.AxisListType
OP = mybir.AluOpType
ACT = mybir.ActivationFunctionType

N_CORES = 8
T, D = 32768, 2048
TL = T // N_CORES          # 4096 time steps per core
NB = 10                    # histogram bins per axis
NPAIR = 4                  # partitions (mask pairs)
J = 2 * NPAIR              # 8 masked-mean columns
NTC = TL // 512            # 8 accumulator groups (512 t each)
NDC = D // 128             # 16 contraction chunks
NCH = TL // 128            # 32 binning chunks of 128 t
MEM = 100
SN = 10

# accumulator tcn -> (bank b, quadrant q): tcn = 3*b + q, q in {0,1,2}
ACC_MAP = [(tcn // 3, tcn % 3) for tcn in range(NTC)]

_CACHE = {}
LAST_RESULTS = None


def _build(debug=False, variant="main"):
    sim1 = variant.startswith("sim1")
    nc = bacc.Bacc("TRN2", target_bir_lowering=False, debug=False,
                   num_devices=1 if sim1 else N_CORES)
    ht = nc.dram_tensor("ht", [D, TL], F32, kind="ExternalInput").ap()
    mmat = nc.dram_tensor("mmat", [D, J], F32, kind="ExternalInput").ap()
    invc = nc.dram_tensor("invc", [128, 1], F32, kind="ExternalInput").ap()
    memt = nc.dram_tensor("memt", [D, MEM], F32, kind="ExternalInput").ap()
    sampt = nc.dram_tensor("sampt", [D, SN], F32, kind="ExternalInput").ap()
    out = nc.dram_tensor("out", [9], F32, kind="ExternalOutput").ap()
    if debug:
        dbg_st = nc.dram_tensor("dbg_st", [J, 512], F32, kind="ExternalOutput").ap()
        dbg_gmm = nc.dram_tensor("dbg_gmm", [J, 2], F32, kind="ExternalOutput").ap()
        dbg_rmat = nc.dram_tensor("dbg_rmat", [J + 1, J], F32, kind="ExternalOutput").ap()
        dbg_bin = nc.dram_tensor("dbg_bin", [128, 16], I32, kind="ExternalOutput").ap()
        dbg_gj = nc.dram_tensor("dbg_gj", [NB, NPAIR * NB], F32, kind="ExternalOutput").ap()
        dbg_mm83 = nc.dram_tensor("dbg_mm83", [J, 9], F32, kind="ExternalOutput").ap()

    rg = [list(range(N_CORES))]

    with tile.TileContext(nc) as tc:
        with tc.tile_pool(name="consts", bufs=1) as consts, \
             tc.tile_pool(name="sb", bufs=1) as sb, \
             tc.tile_pool(name="htp", bufs=2) as htp, \
             tc.tile_pool(name="psA", bufs=3, space="PSUM") as psA_pool, \
             tc.tile_pool(name="psJ", bufs=2, space="PSUM") as psJ_pool, \
             tc.tile_pool(name="misc", bufs=3, space="PSUM") as misc, \
             tc.tile_pool(name="dram", bufs=1, space="DRAM") as dram:

            # ---- constants / small inputs ----
            ident10 = consts.tile([NB, NB], F32, tag="id10")
            make_identity(nc, ident10[:])
            ones128 = consts.tile([128, 1], F32, tag="o128")
            nc.gpsimd.memset(ones128[:], 1.0)
            ones10 = consts.tile([NB, 1], F32, tag="o10")
            nc.gpsimd.memset(ones10[:], 1.0)
            ones1_10 = consts.tile([1, NB], F32, tag="o110")
            nc.gpsimd.memset(ones1_10[:], 1.0)
            ones8x8 = consts.tile([J, J], F32, tag="o88")
            nc.gpsimd.memset(ones8x8[:], 1.0)

            m_sb = consts.tile([128, NDC * J], F32, tag="msb")
            nc.gpsimd.dma_start(
                out=m_sb[:].rearrange("p (k j) -> p k j", j=J),
                in_=mmat.rearrange("(k p) j -> p k j", p=128))
            invc_sb = consts.tile([128, 1], F32, tag="invc")
            nc.gpsimd.dma_start(out=invc_sb[:], in_=invc[:])
            mem_sb = consts.tile([128, NDC * MEM], F32, tag="memsb")
            nc.gpsimd.dma_start(
                out=mem_sb[:].rearrange("p (k f) -> p k f", f=MEM),
                in_=memt.rearrange("(k p) f -> p k f", p=128))
            samp_sb = consts.tile([128, NDC * SN], F32, tag="sampsb")
            nc.gpsimd.dma_start(
                out=samp_sb[:].rearrange("p (k f) -> p k f", f=SN),
                in_=sampt.rearrange("(k p) f -> p k f", p=128))

            # ---- differentiation branch: Gram + row norms (early PE) ----
            psG = misc.tile([SN, SN], F32, tag="m")
            for k in range(NDC):
                nc.tensor.matmul(psG[:], samp_sb[:, k * SN:(k + 1) * SN],
                                 samp_sb[:, k * SN:(k + 1) * SN],
                                 start=(k == 0), stop=(k == NDC - 1))
            sqs = sb.tile([128, NDC * SN], F32, tag="sqs")
            nc.vector.tensor_tensor(sqs[:], samp_sb[:], samp_sb[:], OP.mult)
            psr = misc.tile([SN, 1], F32, tag="m")
            for k in range(NDC):
                nc.tensor.matmul(psr[:], sqs[:, k * SN:(k + 1) * SN],
                                 ones128[:], start=(k == 0),
                                 stop=(k == NDC - 1))
            g_sb = sb.tile([SN, SN], F32, tag="gsb")
            nc.scalar.copy(g_sb[:], psG[:])
            r_sb = sb.tile([SN, 1], F32, tag="rsb")
            nc.scalar.copy(r_sb[:], psr[:])

            # variance branch (DVE; overlaps stream)
            mem3 = mem_sb[:].rearrange("p (k f) -> p k f", f=MEM)
            mean16 = sb.tile([128, NDC], F32, tag="mean16")
            nc.vector.tensor_reduce(mean16[:], mem3, AX.X, OP.add)
            nc.vector.tensor_scalar(mean16[:], mean16[:], 1.0 / MEM, None,
                                    OP.mult)
            cent = sb.tile([128, NDC * MEM], F32, tag="cent")
            nc.vector.tensor_tensor(
                cent[:].rearrange("p (k f) -> p k f", f=MEM), mem3,
                mean16[:, :, None].broadcast_to([128, NDC, MEM]), OP.subtract)
            nc.vector.tensor_tensor(cent[:], cent[:], cent[:], OP.mult)
            var16 = sb.tile([128, NDC], F32, tag="var16")
            nc.vector.tensor_reduce(
                var16[:], cent[:].rearrange("p (k f) -> p k f", f=MEM),
                AX.X, OP.add)
            nc.vector.tensor_scalar(var16[:], var16[:], 1.0 / (MEM - 1), None,
                                    OP.mult)
            redv = sb.tile([128, 1], F32, tag="redv")
            nc.vector.tensor_reduce(redv[:], var16[:], AX.X, OP.add)
            v2 = sb.tile([128, NDC], F32, tag="v2")
            nc.vector.tensor_tensor(v2[:], var16[:], var16[:], OP.mult)
            redv2 = sb.tile([128, 1], F32, tag="redv2")
            nc.vector.tensor_reduce(redv2[:], v2[:], AX.X, OP.add)
            pstv = misc.tile([1, 1], F32, tag="m")
            nc.tensor.matmul(pstv[:], redv[:], ones128[:], start=True,
                             stop=True)
            tv_sb = sb.tile([1, 1], F32, tag="tvsb")
            nc.scalar.copy(tv_sb[:], pstv[:])
            pss2 = misc.tile([1, 1], F32, tag="m")
            nc.tensor.matmul(pss2[:], redv2[:], ones128[:], start=True,
                             stop=True)
            s2_sb = sb.tile([1, 1], F32, tag="s2sb")
            nc.scalar.copy(s2_sb[:], pss2[:])

            tvsq = sb.tile([1, 1], F32, tag="tvsq")
            nc.vector.tensor_tensor(tvsq[:], tv_sb[:], tv_sb[:], OP.mult)
            dden = sb.tile([1, 1], F32, tag="dden")
            nc.vector.scalar_tensor_tensor(dden[:], tvsq[:], 1e-6, s2_sb[:],
                                           OP.mult, OP.add)
            rdden = sb.tile([1, 1], F32, tag="rdden")
            nc.vector.reciprocal(rdden[:], dden[:])
            eff_sb = sb.tile([1, 1], F32, tag="effsb")
            nc.vector.tensor_tensor(eff_sb[:], tvsq[:], rdden[:], OP.mult)

            # cdist tail: d2 = r_i + r_j - 2G
            rrow_ps = misc.tile([1, SN], F32, tag="m")
            nc.tensor.transpose(rrow_ps[:], r_sb[:], ident10[:])
            rrow = sb.tile([1, SN], F32, tag="rrow")
            nc.scalar.copy(rrow[:], rrow_ps[:])
            rB = misc.tile([SN, SN], F32, tag="m")
            nc.tensor.matmul(rB[:], ones1_10[:], rrow[:], start=True,
                             stop=True)
            d2 = sb.tile([SN, SN], F32, tag="d2")
            nc.vector.scalar_tensor_tensor(d2[:], g_sb[:], -2.0, rB[:],
                                           OP.mult, OP.add)
            nc.vector.tensor_scalar(d2[:], d2[:], r_sb[:], 0.0, OP.add,
                                    OP.max)
            dst = sb.tile([SN, SN], F32, tag="dst")
            nc.scalar.activation(dst[:], d2[:], ACT.Sqrt)
            dsum = sb.tile([SN, 1], F32, tag="dsum")
            nc.vector.tensor_reduce(dsum[:], dst[:], AX.X, OP.add)
            psD = misc.tile([1, 1], F32, tag="m")
            nc.tensor.matmul(psD[:], dsum[:], ones10[:], start=True, stop=True)
            avg_sb = sb.tile([1, 1], F32, tag="avgsb")
            nc.vector.tensor_scalar(avg_sb[:], psD[:],
                                    float(1.0 / (SN * (SN - 1) + 1e-6)), None,
                                    OP.mult)
            sqtv = sb.tile([1, 1], F32, tag="sqtv")
            nc.scalar.activation(sqtv[:], tv_sb[:], ACT.Sqrt)
            diff_sb = sb.tile([1, 1], F32, tag="diffsb")
            nc.vector.tensor_tensor(diff_sb[:], sqtv[:], avg_sb[:], OP.mult)
            tanhd = sb.tile([1, 1], F32, tag="tanhd")
            nc.scalar.activation(tanhd[:], diff_sb[:], ACT.Tanh)

            # ---- stage A: stream HT, S.T = M.T @ HT into 3 packed banks ----
            psA = [psA_pool.tile([128, 512], F32, tag="sacc", name=f"psA{i}")
                   for i in range(3)]
            if variant in ("bigdma4", "bigdma8", "sim1big"):
                ng = 8 if variant == "bigdma8" else 4
                per = NDC // ng          # d-chunks per DMA group
                ht3 = ht.rearrange("(c p) t -> p c t", p=128)
                for g in range(ng):
                    htt = htp.tile([128, per * TL], F32, tag="htt",
                                   name="htt")
                    nc.sync.dma_start(
                        out=htt[:].rearrange("p (c t) -> p c t", t=TL),
                        in_=ht3[:, g * per:(g + 1) * per, :])
                    for ci in range(per):
                        dk = g * per + ci
                        for tcn in range(NTC):
                            b, q = ACC_MAP[tcn]
                            nc.tensor.matmul(
                                psA[b][32 * q:32 * q + J, :],
                                m_sb[:, dk * J:(dk + 1) * J],
                                htt[:, ci * TL + tcn * 512:
                                    ci * TL + (tcn + 1) * 512],
                                start=(dk == 0), stop=(dk == NDC - 1))
            else:
                sched = {"main": list(range(NDC)), "ndc1": [0],
                         "stream3": list(range(NDC)) * 3, "sim1": list(range(NDC)),
                         "multiq": list(range(NDC))}[variant]
                qengs = ([nc.sync, nc.scalar, nc.vector, nc.gpsimd]
                         if variant == "multiq" else [nc.sync])
                for i, dk in enumerate(sched):
                    htt = htp.tile([128, TL], F32, tag="htt", name="htt")
                    qengs[i % len(qengs)].dma_start(
                        out=htt[:], in_=ht[dk * 128:(dk + 1) * 128, :])
                    for tcn in range(NTC):
                        b, q = ACC_MAP[tcn]
                        nc.tensor.matmul(psA[b][32 * q:32 * q + J, :],
                                         m_sb[:, dk * J:(dk + 1) * J],
                                         htt[:, tcn * 512:(tcn + 1) * 512],
                                         start=(i == 0),
                                         stop=(i == len(sched) - 1))

            # ---- stage B: scale to SBUF (lane-aligned), min/max, AllReduce --
            # stS layout: [128, 3*512]; group (b,q): rows 32q..32q+7 hold
            # S.T rows for tcn=3b+q at cols b*512.., row 32q+8 = ones.
            stS = sb.tile([128, 3 * 512], F32, tag="sts")
            ones1536 = sb.tile([1, 3 * 512], F32, tag="ones1536")
            nc.vector.memset(ones1536[:], 1.0)
            for q in range(3):
                nc.sync.dma_start(out=stS[32 * q + J:32 * q + J + 1, :],
                                  in_=ones1536[:])
            mxb = sb.tile([128, 3], F32, tag="mxb")
            mnb = sb.tile([128, 3], F32, tag="mnb")
            nc.gpsimd.memset(mxb[:], -3.0e38)
            nc.gpsimd.memset(mnb[:], 3.0e38)
            for tcn in range(NTC):
                b, q = ACC_MAP[tcn]
                seg = stS[32 * q:32 * q + J, b * 512:(b + 1) * 512]
                nc.scalar.mul(seg, psA[b][32 * q:32 * q + J, :],
                              invc_sb[32 * q:32 * q + J, :])
                nc.vector.tensor_reduce(mxb[32 * q:32 * q + J, b:b + 1], seg,
                                        AX.X, OP.max)
                nc.vector.tensor_reduce(mnb[32 * q:32 * q + J, b:b + 1], seg,
                                        AX.X, OP.min)
            # gather lanes {32q+j} -> [8, 3] via SBUF->SBUF DMA remap
            mx83 = sb.tile([J, 9], F32, tag="mx83")
            mn83 = sb.tile([J, 9], F32, tag="mn83")
            for q in range(3):
                nc.sync.dma_start(out=mx83[:, 3 * q:3 * q + 3],
                                  in_=mxb[32 * q:32 * q + J, :])
                nc.sync.dma_start(out=mn83[:, 3 * q:3 * q + 3],
                                  in_=mnb[32 * q:32 * q + J, :])
            minmax = sb.tile([J, 2], F32, tag="minmax")
            nc.vector.tensor_reduce(minmax[:, 0:1], mx83[:], AX.X, OP.max)
            tmn = sb.tile([J, 1], F32, tag="tmn")
            nc.vector.tensor_reduce(tmn[:], mn83[:], AX.X, OP.min)
            nc.vector.tensor_scalar(minmax[:, 1:2], tmn[:], -1.0, None,
                                    OP.mult)
            cbA = dram.tile([J, 2], F32, tag="cba")
            cbB = dram.tile([J, 2], F32, tag="cbb")
            nc.gpsimd.dma_start(out=cbA[:], in_=minmax[:])
            if sim1:
                nc.gpsimd.dma_start(out=cbB[:], in_=cbA[:])
            else:
                nc.gpsimd.collective_compute("AllReduce", OP.max,
                                             replica_groups=rg,
                                             ins=[cbA.opt()],
                                             outs=[cbB.opt()])
            gmm = sb.tile([J, 2], F32, tag="gmm")
            nc.gpsimd.dma_start(out=gmm[:], in_=cbB[:])

            # s1 = 10/(max-min+1e-6); b1 = -min*s1 - 0.5 (RNE cast -> floor)
            gmn = sb.tile([J, 1], F32, tag="gmn")
            nc.vector.tensor_scalar(gmn[:], gmm[:, 1:2], -1.0, None, OP.mult)
            dden2 = sb.tile([J, 1], F32, tag="dden2")
            nc.vector.tensor_tensor(dden2[:], gmm[:, 0:1], gmn[:], OP.subtract)
            nc.vector.tensor_scalar(dden2[:], dden2[:], 1e-6, None, OP.add)
            rdd = sb.tile([J, 1], F32, tag="rdd")
            nc.vector.reciprocal(rdd[:], dden2[:])
            s1 = sb.tile([J, 1], F32, tag="s1")
            nc.vector.tensor_scalar(s1[:], rdd[:], 10.0, None, OP.mult)
            b1 = sb.tile([J, 1], F32, tag="b1")
            nc.vector.tensor_tensor(b1[:], gmn[:], s1[:], OP.mult)
            nc.vector.tensor_scalar(b1[:], b1[:], -1.0, -0.5, OP.mult, OP.add)

            # R [9,8] replicated at partition bases 0/32/64:
            # rows 32q..32q+7 diag(s1), row 32q+8 = b1 row
            s1b = sb.tile([J, J], F32, tag="s1b")
            nc.vector.tensor_scalar(s1b[:], ones8x8[:], s1[:], None, OP.mult)
            rmat = sb.tile([128, J], F32, tag="rmat")
            nc.gpsimd.memset(rmat[:], 0.0)
            nc.gpsimd.affine_select(out=rmat[0:J, :], in_=s1b[:],
                                    compare_op=OP.is_equal, fill=0.0, base=0,
                                    pattern=[[-1, J]], channel_multiplier=1)
            nc.sync.dma_start(out=rmat[J:J + 1, 0:J], in_=b1[:])
            nc.sync.dma_start(out=rmat[32:32 + J + 1, :], in_=rmat[0:J + 1, :])
            nc.sync.dma_start(out=rmat[64:64 + J + 1, :], in_=rmat[0:J + 1, :])

            # ---- stage C: affine+transpose via PE, bin, one-hot, joints ----
            psC = misc.tile([128, NCH * J], F32, tag="m")
            for tcn in range(NTC):
                b, q = ACC_MAP[tcn]
                for c in range(4):
                    gc = tcn * 4 + c
                    nc.tensor.matmul(
                        psC[:, gc * J:(gc + 1) * J],
                        stS[32 * q:32 * q + J + 1,
                            b * 512 + c * 128:b * 512 + (c + 1) * 128],
                        rmat[32 * q:32 * q + J + 1, :],
                        start=True, stop=True)
            binint = sb.tile([128, NCH * J], I32, tag="binint")
            nc.vector.tensor_copy(binint[:], psC[:])
            nc.vector.tensor_scalar(binint[:], binint[:], 0, NB - 1, OP.max,
                                    OP.min)
            ohsb = sb.tile([128, NCH * J * NB], F32, tag="ohsb")
            oh3 = ohsb[:].rearrange("p (c b) -> p c b", b=NB)
            for b in range(NB):
                nc.vector.tensor_scalar(oh3[:, :, b], binint[:], b, None,
                                        OP.is_equal)
            # joint histograms: psJt1 packs pairs 0..2 at bases 0/32/64
            psJt1 = psJ_pool.tile([128, NB], F32, tag="pj", name="psJt1")
            psJt2 = psJ_pool.tile([NB, NB], F32, tag="pj", name="psJt2")
            for p in range(NPAIR):
                outap = (psJt2[:] if p == 3
                         else psJt1[32 * p:32 * p + NB, :])
                for c in range(NCH):
                    xa = (c * J + 2 * p) * NB
                    ya = (c * J + 2 * p + 1) * NB
                    nc.tensor.matmul(outap, ohsb[:, xa:xa + NB],
                                     ohsb[:, ya:ya + NB], start=(c == 0),
                                     stop=(c == NCH - 1))
            jm1 = sb.tile([128, NB], F32, tag="jm1")
            jm2 = sb.tile([NB, NB], F32, tag="jm2")
            for p in range(3):
                nc.scalar.copy(jm1[32 * p:32 * p + NB, :],
                               psJt1[32 * p:32 * p + NB, :])
            nc.scalar.copy(jm2[:], psJt2[:])
            cbj = dram.tile([NPAIR, NB * NB], F32, tag="cbj")
            cbj2 = dram.tile([NPAIR, NB * NB], F32, tag="cbj2")
            for p in range(3):
                nc.gpsimd.dma_start(
                    out=cbj[p:p + 1, :],
                    in_=jm1[32 * p:32 * p + NB, :])
            nc.gpsimd.dma_start(out=cbj[3:4, :], in_=jm2[:])
            if sim1:
                nc.gpsimd.dma_start(out=cbj2[:], in_=cbj[:])
            else:
                nc.gpsimd.collective_compute("AllReduce", OP.add,
                                             replica_groups=rg,
                                             ins=[cbj.opt()],
                                             outs=[cbj2.opt()])
            gj = sb.tile([NB, NPAIR * NB], F32, tag="gj")
            nc.gpsimd.dma_start(
                out=gj[:].rearrange("a (p b) -> a p b", b=NB),
                in_=cbj2[:].rearrange("p (a b) -> a p b", a=NB))

            # ---- stage D: MI per pair ----
            mirow = sb.tile([1, NPAIR], F32, tag="mirow")
            for p in range(NPAIR):
                gjp = gj[:, p * NB:(p + 1) * NB]
                rowsum = sb.tile([NB, 1], F32, tag="rowsum", name="rowsum")
                nc.vector.tensor_reduce(rowsum[:], gjp, AX.X, OP.add)
                colps = misc.tile([NB, 1], F32, tag="m", name="colps")
                nc.tensor.matmul(colps[:], gjp, ones10[:], start=True,
                                 stop=True)
                totps = misc.tile([1, 1], F32, tag="m", name="totps")
                nc.tensor.matmul(totps[:], rowsum[:], ones10[:], start=True,
                                 stop=True)
                tot = sb.tile([1, 1], F32, tag="tot", name="tot")
                nc.vector.tensor_scalar(tot[:], totps[:], 1e-10, None, OP.add)
                tinv = sb.tile([1, 1], F32, tag="tinv", name="tinv")
                nc.vector.reciprocal(tinv[:], tot[:])
                t10ps = misc.tile([NB, 1], F32, tag="m", name="t10ps")
                nc.tensor.matmul(t10ps[:], ones1_10[:], tinv[:], start=True,
                                 stop=True)
                t10 = sb.tile([NB, 1], F32, tag="t10", name="t10")
                nc.scalar.copy(t10[:], t10ps[:])
                jn = sb.tile([NB, NB], F32, tag="jn", name="jn")
                nc.vector.tensor_scalar(jn[:], gjp, t10[:], None, OP.mult)
                px = sb.tile([NB, 1], F32, tag="px", name="px")
                nc.vector.tensor_scalar(px[:], rowsum[:], t10[:], None,
                                        OP.mult)
                py = sb.tile([NB, 1], F32, tag="py", name="py")
                nc.vector.tensor_scalar(py[:], colps[:], t10[:], None,
                                        OP.mult)
                pyr_ps = misc.tile([1, NB], F32, tag="m", name="pyr_ps")
                nc.tensor.transpose(pyr_ps[:], py[:], ident10[:])
                pyr = sb.tile([1, NB], F32, tag="pyr", name="pyr")
                nc.scalar.copy(pyr[:], pyr_ps[:])
                pyB = misc.tile([NB, NB], F32, tag="m", name="pyB")
                nc.tensor.matmul(pyB[:], ones1_10[:], pyr[:], start=True,
                                 stop=True)
                outer = sb.tile([NB, NB], F32, tag="outer", name="outer")
                nc.vector.tensor_scalar(outer[:], pyB[:], px[:], None,
                                        OP.mult)
                num = sb.tile([NB, NB], F32, tag="num", name="num")
                nc.vector.tensor_scalar(num[:], jn[:], 1e-10, None, OP.add)
                nc.vector.tensor_scalar(outer[:], outer[:], 1e-10, None,
                                        OP.add)
                rout = sb.tile([NB, NB], F32, tag="rout", name="rout")
                nc.vector.reciprocal(rout[:], outer[:])
                nc.vector.tensor_tensor(num[:], num[:], rout[:], OP.mult)
                lg = sb.tile([NB, NB], F32, tag="lg", name="lg")
                nc.scalar.activation(lg[:], num[:], ACT.Ln)
                nc.vector.tensor_tensor(lg[:], jn[:], lg[:], OP.mult)
                ms = sb.tile([NB, 1], F32, tag="ms", name="ms")
                nc.vector.tensor_reduce(ms[:], lg[:], AX.X, OP.add)
                mips = misc.tile([1, 1], F32, tag="m", name="mips")
                nc.tensor.matmul(mips[:], ms[:], ones10[:], start=True,
                                 stop=True)
                nc.vector.tensor_scalar(mirow[:, p:p + 1], mips[:], 0.0, None,
                                        OP.max)

            integ = sb.tile([1, 1], F32, tag="integ")
            nc.vector.tensor_reduce(integ[:], mirow[:], AX.X, OP.min)
            consc = sb.tile([1, 1], F32, tag="consc")
            nc.vector.tensor_tensor(consc[:], integ[:], tanhd[:], OP.add)

            outrow = sb.tile([1, 9], F32, tag="outrow")
            nc.vector.tensor_copy(outrow[:, 0:1], consc[:])
            nc.vector.tensor_copy(outrow[:, 1:2], diff_sb[:])
            nc.vector.tensor_copy(outrow[:, 2:3], eff_sb[:])
            nc.vector.tensor_copy(outrow[:, 3:4], tv_sb[:])
            nc.vector.tensor_copy(outrow[:, 4:5], integ[:])
            nc.vector.tensor_copy(outrow[:, 5:9], mirow[:])
            nc.sync.dma_start(out=out[:], in_=outrow[:])
            if debug:
                nc.sync.dma_start(out=dbg_st[:], in_=stS[0:J, 0:512])
                nc.sync.dma_start(out=dbg_gmm[:], in_=gmm[:])
                nc.sync.dma_start(out=dbg_rmat[:], in_=rmat[0:J + 1, :])
                nc.sync.dma_start(out=dbg_bin[:], in_=binint[:, 0:16])
                nc.sync.dma_start(out=dbg_gj[:], in_=gj[:])
                nc.sync.dma_start(out=dbg_mm83[:], in_=mx83[:])

    nc.compile()
    return nc


def _build_variant(name):
    return _build(variant=name)


def _get_nc(debug=False):
    key = ("ncd" if debug else "nc")
    if key not in _CACHE:
        _CACHE[key] = _build(debug)
    return _CACHE[key]


def kernel(state, state_memory, state_history, partitions, sample_idx,
           trace=False, debug=False):
    global LAST_RESULTS
    state = np.asarray(state, np.float32)
    state_memory = np.asarray(state_memory, np.float32)
    state_history = np.asarray(state_history, np.float32)
    partitions = np.asarray(partitions)
    sample_idx = np.asarray(sample_idx)

    mmat = np.empty((D, J), np.float32)
    invc8 = np.empty((J,), np.float32)
    pf = partitions.astype(np.float32)
    for p in range(NPAIR):
        mmat[:, 2 * p] = pf[p]
        mmat[:, 2 * p + 1] = np.float32(1.0) - pf[p]
        invc8[2 * p] = np.float32(1.0) / pf[p].sum(dtype=np.float32)
        invc8[2 * p + 1] = np.float32(1.0) / (np.float32(1.0) - pf[p]).sum(
            dtype=np.float32)
    invc = np.zeros((128, 1), np.float32)
    for q in range(3):
        invc[32 * q:32 * q + J, 0] = invc8
    memory = np.concatenate([state, state_memory[state.shape[0]:]], axis=0)
    memt = np.ascontiguousarray(memory.T)
    sampt = np.ascontiguousarray(memory[sample_idx].T)

    in_maps = []
    for c in range(N_CORES):
        htc = np.ascontiguousarray(state_history[c * TL:(c + 1) * TL, :].T)
        in_maps.append({"ht": htc, "mmat": mmat, "invc": invc,
                        "memt": memt, "sampt": sampt})

    nc = _get_nc(debug)
    res = run_bass_kernel_spmd(nc, in_maps, list(range(N_CORES)),
                               trace=trace)
    LAST_RESULTS = res
    return np.asarray(res.results[0]["out"], np.float32)



# revision 13
# speedup vs baseline: 1.3919x; 1.3919x over previous
"""Trainium2 Bass kernel for nn_ConsciousnessMonitor (histogram_binning).

kernel(**inputs) takes FULL unsharded numpy inputs, returns the full (9,)
float32 output. Shards state_history along time across 8 NeuronCores:
masked means via PE matmul while streaming (ht chunks stationary, mask
columns moving, so S arrives time-major), min/max + joint-histogram MI
with two small AllReduces, differentiation branch replicated per core.

Self-contained: shapes/sharding hardcoded; reads no sibling files.
"""
import numpy as np

import concourse.bacc as bacc
import concourse.tile as tile
import concourse.mybir as mybir
from concourse.bass_utils import run_bass_kernel_spmd
from concourse.masks import make_identity

F32 = mybir.dt.float32
I32 = mybir.dt.int32
AX = mybir.AxisListType
OP = mybir.AluOpType
ACT = mybir.ActivationFunctionType

N_CORES = 8
T, D = 32768, 2048
TL = T // N_CORES          # 4096 time steps per core
NB = 10                    # histogram bins per axis
NPAIR = 4                  # partitions (mask pairs)
J = 2 * NPAIR              # 8 masked-mean columns
NDC = D // 128             # 16 contraction chunks
NCH = TL // 128            # 32 time chunks of 128 (PSUM cols / binning)
MEM = 100
SN = 10

LN_T = float(np.log(np.float32(T)))
INV_T = 1.0 / T
EPS_N = T * 1e-10          # joint-count epsilon under common denominator
EPS_RC = float(T) * T * 1e-10  # outer-product epsilon likewise

_CACHE = {}
LAST_RESULTS = None


def _build(debug=False, variant="main"):
    sim1 = variant.startswith("sim1")
    nc = bacc.Bacc("TRN2", target_bir_lowering=False, debug=False,
                   num_devices=1 if sim1 else N_CORES)
    ht = nc.dram_tensor("ht", [D, TL], F32, kind="ExternalInput").ap()
    mmat = nc.dram_tensor("mmat", [128, NDC * J], F32,
                          kind="ExternalInput").ap()
    invc = nc.dram_tensor("invc", [J, 1], F32, kind="ExternalInput").ap()
    memt = nc.dram_tensor("memt", [128, NDC * MEM], F32,
                          kind="ExternalInput").ap()
    sampt = nc.dram_tensor("sampt", [128, NDC * SN], F32,
                           kind="ExternalInput").ap()
    out = nc.dram_tensor("out", [9], F32, kind="ExternalOutput").ap()
    if debug:
        dbg_st = nc.dram_tensor("dbg_st", [128, J], F32,
                                kind="ExternalOutput").ap()
        dbg_gmm = nc.dram_tensor("dbg_gmm", [J, 2], F32,
                                 kind="ExternalOutput").ap()
        dbg_s1b1 = nc.dram_tensor("dbg_s1b1", [J, 2], F32,
                                  kind="ExternalOutput").ap()
        dbg_bin = nc.dram_tensor("dbg_bin", [128, 16], I32,
                                 kind="ExternalOutput").ap()
        dbg_gj = nc.dram_tensor("dbg_gj", [NB, NPAIR * NB], F32,
                                kind="ExternalOutput").ap()

    rg = [list(range(N_CORES))]

    with tile.TileContext(nc) as tc:
        with tc.tile_pool(name="consts", bufs=1) as consts, \
             tc.tile_pool(name="sb", bufs=1) as sb, \
             tc.tile_pool(name="htp", bufs=2) as htp, \
             tc.tile_pool(name="psA", bufs=1, space="PSUM") as psA_pool, \
             tc.tile_pool(name="psJ", bufs=1, space="PSUM") as psJ_pool, \
             tc.tile_pool(name="misc", bufs=3, space="PSUM") as misc, \
             tc.tile_pool(name="dram", bufs=1, space="DRAM") as dram:

            # ---- constants / small inputs ----
            ident10 = consts.tile([NB, NB], F32, tag="id10")
            make_identity(nc, ident10[:])
            ident128 = consts.tile([128, 128], F32, tag="id128")
            make_identity(nc, ident128[:])
            ones128 = consts.tile([128, 1], F32, tag="o128")
            nc.gpsimd.memset(ones128[:], 1.0)
            ones10 = consts.tile([NB, 1], F32, tag="o10")
            nc.gpsimd.memset(ones10[:], 1.0)
            ones1_10 = consts.tile([1, NB], F32, tag="o110")
            nc.gpsimd.memset(ones1_10[:], 1.0)
            ones1_128 = consts.tile([1, 128], F32, tag="o1128")
            nc.gpsimd.memset(ones1_128[:], 1.0)

            # preload the Ln activation table early (Sqrt/Tanh load via the
            # differentiation branch) so no table load lands on the tail
            lnwarm = sb.tile([1, 1], F32, tag="lnwarm")
            nc.scalar.activation(lnwarm[:], ones10[0:1, :1], ACT.Ln)

            m_sb = consts.tile([128, NDC * J], F32, tag="msb")
            nc.sync.dma_start(out=m_sb[:], in_=mmat[:])
            invc_sb = consts.tile([J, 1], F32, tag="invc")
            nc.sync.dma_start(out=invc_sb[:], in_=invc[:])
            mem_sb = consts.tile([128, NDC * MEM], F32, tag="memsb")
            nc.gpsimd.dma_start(out=mem_sb[:], in_=memt[:])
            samp_sb = consts.tile([128, NDC * SN], F32, tag="sampsb")
            nc.gpsimd.dma_start(out=samp_sb[:], in_=sampt[:])

            # ---- differentiation branch (all early; overlaps stream) ----
            psG = misc.tile([SN, SN], F32, tag="m")
            for k in range(NDC):
                nc.tensor.matmul(psG[:], samp_sb[:, k * SN:(k + 1) * SN],
                                 samp_sb[:, k * SN:(k + 1) * SN],
                                 start=(k == 0), stop=(k == NDC - 1))
            sqs = sb.tile([128, NDC * SN], F32, tag="sqs")
            nc.vector.tensor_tensor(sqs[:], samp_sb[:], samp_sb[:], OP.mult)
            psr = misc.tile([SN, 1], F32, tag="m")
            for k in range(NDC):
                nc.tensor.matmul(psr[:], sqs[:, k * SN:(k + 1) * SN],
                                 ones128[:], start=(k == 0),
                                 stop=(k == NDC - 1))
            g_sb = sb.tile([SN, SN], F32, tag="gsb")
            nc.scalar.copy(g_sb[:], psG[:])
            r_sb = sb.tile([SN, 1], F32, tag="rsb")
            nc.scalar.copy(r_sb[:], psr[:])

            # variance branch (DVE; early)
            mem3 = mem_sb[:].rearrange("p (k f) -> p k f", f=MEM)
            mean16 = sb.tile([128, NDC], F32, tag="mean16")
            nc.vector.tensor_reduce(mean16[:], mem3, AX.X, OP.add)
            nc.vector.tensor_scalar(mean16[:], mean16[:], 1.0 / MEM, None,
                                    OP.mult)
            cent = sb.tile([128, NDC * MEM], F32, tag="cent")
            nc.vector.tensor_tensor(
                cent[:].rearrange("p (k f) -> p k f", f=MEM), mem3,
                mean16[:, :, None].broadcast_to([128, NDC, MEM]), OP.subtract)
            nc.vector.tensor_tensor(cent[:], cent[:], cent[:], OP.mult)
            var16 = sb.tile([128, NDC], F32, tag="var16")
            nc.vector.tensor_reduce(
                var16[:], cent[:].rearrange("p (k f) -> p k f", f=MEM),
                AX.X, OP.add)
            nc.vector.tensor_scalar(var16[:], var16[:], 1.0 / (MEM - 1), None,
                                    OP.mult)
            redv = sb.tile([128, 1], F32, tag="redv")
            nc.vector.tensor_reduce(redv[:], var16[:], AX.X, OP.add)
            v2 = sb.tile([128, NDC], F32, tag="v2")
            nc.vector.tensor_tensor(v2[:], var16[:], var16[:], OP.mult)
            redv2 = sb.tile([128, 1], F32, tag="redv2")
            nc.vector.tensor_reduce(redv2[:], v2[:], AX.X, OP.add)
            pstv = misc.tile([1, 1], F32, tag="m")
            nc.tensor.matmul(pstv[:], redv[:], ones128[:], start=True,
                             stop=True)
            tv_sb = sb.tile([1, 1], F32, tag="tvsb")
            nc.scalar.copy(tv_sb[:], pstv[:])
            pss2 = misc.tile([1, 1], F32, tag="m")
            nc.tensor.matmul(pss2[:], redv2[:], ones128[:], start=True,
                             stop=True)
            s2_sb = sb.tile([1, 1], F32, tag="s2sb")
            nc.scalar.copy(s2_sb[:], pss2[:])

            tvsq = sb.tile([1, 1], F32, tag="tvsq")
            nc.vector.tensor_tensor(tvsq[:], tv_sb[:], tv_sb[:], OP.mult)
            dden = sb.tile([1, 1], F32, tag="dden")
            nc.vector.scalar_tensor_tensor(dden[:], tvsq[:], 1e-6, s2_sb[:],
                                           OP.mult, OP.add)
            rdden = sb.tile([1, 1], F32, tag="rdden")
            nc.vector.reciprocal(rdden[:], dden[:])
            eff_sb = sb.tile([1, 1], F32, tag="effsb")
            nc.vector.tensor_tensor(eff_sb[:], tvsq[:], rdden[:], OP.mult)

            # cdist tail: d2 = r_i + r_j - 2G
            rrow_ps = misc.tile([1, SN], F32, tag="m")
            nc.tensor.transpose(rrow_ps[:], r_sb[:], ident10[:])
            rrow = sb.tile([1, SN], F32, tag="rrow")
            nc.scalar.copy(rrow[:], rrow_ps[:])
            rB = misc.tile([SN, SN], F32, tag="m")
            nc.tensor.matmul(rB[:], ones1_10[:], rrow[:], start=True,
                             stop=True)
            d2 = sb.tile([SN, SN], F32, tag="d2")
            nc.vector.scalar_tensor_tensor(d2[:], g_sb[:], -2.0, rB[:],
                                           OP.mult, OP.add)
            nc.vector.tensor_scalar(d2[:], d2[:], r_sb[:], 0.0, OP.add,
                                    OP.max)
            dst = sb.tile([SN, SN], F32, tag="dst")
            nc.scalar.activation(dst[:], d2[:], ACT.Sqrt)
            dsum = sb.tile([SN, 1], F32, tag="dsum")
            nc.vector.tensor_reduce(dsum[:], dst[:], AX.X, OP.add)
            psD = misc.tile([1, 1], F32, tag="m")
            nc.tensor.matmul(psD[:], dsum[:], ones10[:], start=True, stop=True)
            avg_sb = sb.tile([1, 1], F32, tag="avgsb")
            nc.vector.tensor_scalar(avg_sb[:], psD[:],
                                    float(1.0 / (SN * (SN - 1) + 1e-6)), None,
                                    OP.mult)
            sqtv = sb.tile([1, 1], F32, tag="sqtv")
            nc.scalar.activation(sqtv[:], tv_sb[:], ACT.Sqrt)
            diff_sb = sb.tile([1, 1], F32, tag="diffsb")
            nc.vector.tensor_tensor(diff_sb[:], sqtv[:], avg_sb[:], OP.mult)
            tanhd = sb.tile([1, 1], F32, tag="tanhd")
            nc.scalar.activation(tanhd[:], diff_sb[:], ACT.Tanh)

            # ---- stage A: stream HT; ht chunks stationary, masks moving ----
            # psAll[:, c*J+j] accumulates S.T[t, j] for t-chunk c: 128 t rows
            # on partitions, all 32 chunks x 8 series in half a PSUM bank.
            psAll = psA_pool.tile([128, NCH * J], F32, tag="sacc")
            for dk in range(NDC):
                htt = htp.tile([128, TL], F32, tag="htt", name="htt")
                q = nc.sync if (dk % 2 == 0) else nc.gpsimd
                q.dma_start(out=htt[:], in_=ht[dk * 128:(dk + 1) * 128, :])
                for c in range(NCH):
                    # start zeroes the whole 2KB zero-region (bank), so only
                    # the very first matmul in the bank may carry start=True
                    nc.tensor.matmul(psAll[:, c * J:(c + 1) * J],
                                     htt[:, c * 128:(c + 1) * 128],
                                     m_sb[:, dk * J:(dk + 1) * J],
                                     start=(dk == 0 and c == 0),
                                     stop=(dk == NDC - 1 and c == NCH - 1),
                                     skip_group_check=True)

            # ---- stage B: raw min/max per series, scale, AllReduce(max) ----
            ps3 = psAll[:].rearrange("p (c j) -> p j c", j=J)
            mx8 = sb.tile([128, J], F32, tag="mx8")
            mn8 = sb.tile([128, J], F32, tag="mn8")
            nc.vector.tensor_reduce(mx8[:], ps3, AX.X, OP.max)
            nc.vector.tensor_reduce(mn8[:], ps3, AX.X, OP.min)
            psmx = misc.tile([J, 128], F32, tag="m", name="psmx")
            nc.tensor.transpose(psmx[:], mx8[:], ident128[:])
            psmn = misc.tile([J, 128], F32, tag="m", name="psmn")
            nc.tensor.transpose(psmn[:], mn8[:], ident128[:])
            minmax = sb.tile([J, 2], F32, tag="minmax")
            # scaled max = invc * raw max ; second col = -(invc * raw min)
            tmx = sb.tile([J, 1], F32, tag="tmx")
            nc.vector.tensor_reduce(tmx[:], psmx[:], AX.X, OP.max)
            tmn = sb.tile([J, 1], F32, tag="tmn")
            nc.vector.tensor_reduce(tmn[:], psmn[:], AX.X, OP.min)
            nc.vector.tensor_scalar(minmax[:, 0:1], tmx[:], invc_sb[:], None,
                                    OP.mult)
            nc.vector.tensor_scalar(minmax[:, 1:2], tmn[:], invc_sb[:], -1.0,
                                    OP.mult, OP.mult)
            cbA = dram.tile([J, 2], F32, tag="cba")
            cbB = dram.tile([J, 2], F32, tag="cbb")
            nc.sync.dma_start(out=cbA[:], in_=minmax[:])
            if sim1:
                nc.sync.dma_start(out=cbB[:], in_=cbA[:])
            else:
                nc.gpsimd.collective_compute("AllReduce", OP.max,
                                             replica_groups=rg,
                                             ins=[cbA.opt()],
                                             outs=[cbB.opt()])
            gmm = sb.tile([J, 2], F32, tag="gmm")
            nc.sync.dma_start(out=gmm[:], in_=cbB[:])

            # s1 = 10/(max-min+1e-6); b1 = -min*s1 - 0.5 (RNE cast -> floor)
            # on raw S: s1eff = s1*invc (gmm holds scaled max / -scaled min)
            rng8 = sb.tile([J, 1], F32, tag="rng8")
            nc.vector.tensor_reduce(rng8[:], gmm[:], AX.X, OP.add)
            nc.vector.tensor_scalar(rng8[:], rng8[:], 1e-6, None, OP.add)
            rdd = sb.tile([J, 1], F32, tag="rdd")
            nc.vector.reciprocal(rdd[:], rng8[:])
            sbcol = sb.tile([J, 2], F32, tag="sbcol")
            # col0: s1eff = 10*invc/(range) ; col1: b1 = gmm[:,1]*s1 - 0.5
            s1 = sb.tile([J, 1], F32, tag="s1")
            nc.vector.tensor_scalar(s1[:], rdd[:], 10.0, None, OP.mult)
            nc.vector.tensor_scalar(sbcol[:, 0:1], s1[:], invc_sb[:], None,
                                    OP.mult)
            nc.vector.tensor_tensor(sbcol[:, 1:2], gmm[:, 1:2], s1[:],
                                    OP.mult)
            nc.vector.tensor_scalar(sbcol[:, 1:2], sbcol[:, 1:2], -0.5, None,
                                    OP.add)
            # broadcast [J,2] -> rows: transpose then K=1 matmul -> [128, J]
            psTs = misc.tile([1, J], F32, tag="m", name="psTs")
            nc.tensor.transpose(psTs[:], sbcol[:, 0:1], ident10[0:J, 0:J])
            psTb = misc.tile([1, J], F32, tag="m", name="psTb")
            nc.tensor.transpose(psTb[:], sbcol[:, 1:2], ident10[0:J, 0:J])
            rowsb = sb.tile([1, 2 * J], F32, tag="rowsb")
            nc.scalar.copy(rowsb[:, 0:J], psTs[:])
            nc.scalar.copy(rowsb[:, J:2 * J], psTb[:])
            psbc = misc.tile([128, 2 * J], F32, tag="m", name="psbc")
            nc.tensor.matmul(psbc[:, 0:J], ones1_128[:], rowsb[:, 0:J],
                             start=True, stop=False, skip_group_check=True)
            nc.tensor.matmul(psbc[:, J:2 * J], ones1_128[:],
                             rowsb[:, J:2 * J], start=False, stop=True,
                             skip_group_check=True)
            s1rep = sb.tile([128, 2 * J], F32, tag="s1rep")
            nc.scalar.copy(s1rep[:], psbc[:])

            # ---- stage C: affine + int-cast + clamp + one-hot + joints ----
            binf = sb.tile([128, NCH * J], F32, tag="binf")
            b3 = binf[:].rearrange("p (c j) -> p c j", j=J)
            nc.vector.tensor_tensor(
                b3, psAll[:].rearrange("p (c j) -> p c j", j=J),
                s1rep[:, None, 0:J].broadcast_to([128, NCH, J]), OP.mult)
            nc.vector.tensor_tensor(
                b3, b3, s1rep[:, None, J:2 * J].broadcast_to([128, NCH, J]),
                OP.add)
            binint = sb.tile([128, NCH * J], I32, tag="binint")
            nc.vector.tensor_copy(binint[:], binf[:])
            nc.vector.tensor_scalar(binint[:], binint[:], 0, NB - 1, OP.max,
                                    OP.min)
            ohsb = sb.tile([128, NCH * J * NB], F32, tag="ohsb")
            oh3 = ohsb[:].rearrange("p (c b) -> p c b", b=NB)
            for b in range(NB):
                eng = nc.vector if b % 2 == 0 else nc.gpsimd
                eng.tensor_scalar(oh3[:, :, b], binint[:], b, None,
                                  OP.is_equal)
            # joint histograms: all 4 pairs side by side in one PSUM bank
            psJt = psJ_pool.tile([NB, NPAIR * NB], F32, tag="pj")
            for c in range(NCH):
                for p in range(NPAIR):
                    xa = (c * J + 2 * p) * NB
                    ya = (c * J + 2 * p + 1) * NB
                    nc.tensor.matmul(psJt[:, p * NB:(p + 1) * NB],
                                     ohsb[:, xa:xa + NB],
                                     ohsb[:, ya:ya + NB],
                                     start=(c == 0 and p == 0),
                                     stop=(c == NCH - 1 and p == NPAIR - 1),
                                     skip_group_check=True)
            gjl = sb.tile([NB, NPAIR * NB], F32, tag="gjl")
            nc.scalar.copy(gjl[:], psJt[:])
            cbj = dram.tile([NB, NPAIR * NB], F32, tag="cbj")
            cbj2 = dram.tile([NB, NPAIR * NB], F32, tag="cbj2")
            nc.sync.dma_start(out=cbj[:], in_=gjl[:])
            if sim1:
                nc.sync.dma_start(out=cbj2[:], in_=cbj[:])
            else:
                nc.gpsimd.collective_compute("AllReduce", OP.add,
                                             replica_groups=rg,
                                             ins=[cbj.opt()],
                                             outs=[cbj2.opt()])
            gj = sb.tile([NB, NPAIR * NB], F32, tag="gj")
            nc.sync.dma_start(out=gj[:], in_=cbj2[:])

            # ---- stage D: batched MI over the 4 pairs ----
            # mi_p = (1/T) sum_ij n_ij*(ln(n_ij+EPS_N)+LN_T-ln(r_i*c_j+EPS_RC))
            gj3 = gj[:].rearrange("a (p b) -> a p b", b=NB)
            r4 = sb.tile([NB, NPAIR], F32, tag="r4")
            nc.vector.tensor_reduce(r4[:], gj3, AX.X, OP.add)
            psc1 = misc.tile([1, NPAIR * NB], F32, tag="m", name="psc1")
            nc.tensor.matmul(psc1[:], ones10[:], gj[:], start=True, stop=True)
            c1 = sb.tile([1, NPAIR * NB], F32, tag="c1")
            nc.scalar.copy(c1[:], psc1[:])
            pscB = misc.tile([NB, NPAIR * NB], F32, tag="m", name="pscB")
            nc.tensor.matmul(pscB[:], ones1_10[:], c1[:], start=True,
                             stop=True)
            rc = sb.tile([NB, NPAIR * NB], F32, tag="rc")
            nc.vector.tensor_tensor(
                rc[:].rearrange("a (p b) -> a p b", b=NB),
                pscB[:].rearrange("a (p b) -> a p b", b=NB),
                r4[:, :, None].broadcast_to([NB, NPAIR, NB]), OP.mult)
            nc.vector.tensor_scalar(rc[:], rc[:], EPS_RC, None, OP.add)
            lnrc = sb.tile([NB, NPAIR * NB], F32, tag="lnrc")
            nc.scalar.activation(lnrc[:], rc[:], ACT.Ln)
            npl = sb.tile([NB, NPAIR * NB], F32, tag="npl")
            nc.vector.tensor_scalar(npl[:], gj[:], EPS_N, None, OP.add)
            lnn = sb.tile([NB, NPAIR * NB], F32, tag="lnn")
            nc.scalar.activation(lnn[:], npl[:], ACT.Ln)
            lterm = sb.tile([NB, NPAIR * NB], F32, tag="lterm")
            nc.vector.scalar_tensor_tensor(lterm[:], lnn[:], LN_T, lnrc[:],
                                           OP.add, OP.subtract)
            nc.vector.tensor_tensor(lterm[:], gj[:], lterm[:], OP.mult)
            rsum = sb.tile([NB, NPAIR], F32, tag="rsum")
            nc.vector.tensor_reduce(
                rsum[:], lterm[:].rearrange("a (p b) -> a p b", b=NB),
                AX.X, OP.add)
            psmi = misc.tile([1, NPAIR], F32, tag="m", name="psmi")
            nc.tensor.matmul(psmi[:], ones10[:], rsum[:], start=True,
                             stop=True)
            mirow = sb.tile([1, NPAIR], F32, tag="mirow")
            nc.vector.tensor_scalar(mirow[:], psmi[:], INV_T, 0.0, OP.mult,
                                    OP.max)

            integ = sb.tile([1, 1], F32, tag="integ")
            nc.vector.tensor_reduce(integ[:], mirow[:], AX.X, OP.min)
            consc = sb.tile([1, 1], F32, tag="consc")
            nc.vector.tensor_tensor(consc[:], integ[:], tanhd[:], OP.add)

            outrow = sb.tile([1, 9], F32, tag="outrow")
            nc.vector.tensor_copy(outrow[:, 0:1], consc[:])
            nc.vector.tensor_copy(outrow[:, 1:2], diff_sb[:])
            nc.vector.tensor_copy(outrow[:, 2:3], eff_sb[:])
            nc.vector.tensor_copy(outrow[:, 3:4], tv_sb[:])
            nc.vector.tensor_copy(outrow[:, 4:5], integ[:])
            nc.vector.tensor_copy(outrow[:, 5:9], mirow[:])
            nc.sync.dma_start(out=out[:], in_=outrow[:])
            if debug:
                nc.sync.dma_start(out=dbg_st[:], in_=binf[:, 0:J])
                nc.sync.dma_start(out=dbg_gmm[:], in_=gmm[:])
                nc.sync.dma_start(out=dbg_s1b1[:], in_=sbcol[:])
                nc.sync.dma_start(out=dbg_bin[:], in_=binint[:, 0:16])
                nc.sync.dma_start(out=dbg_gj[:], in_=gj[:])

    nc.compile()
    return nc


def _build_variant(name):
    return _build(variant=name)


def _get_nc(debug=False):
    key = ("ncd" if debug else "nc")
    if key not in _CACHE:
        _CACHE[key] = _build(debug)
    return _CACHE[key]


def kernel(state, state_memory, state_history, partitions, sample_idx,
           trace=False, debug=False):
    global LAST_RESULTS
    state = np.asarray(state, np.float32)
    state_memory = np.asarray(state_memory, np.float32)
    state_history = np.asarray(state_history, np.float32)
    partitions = np.asarray(partitions)
    sample_idx = np.asarray(sample_idx)

    mmat = np.empty((D, J), np.float32)
    invc8 = np.empty((J,), np.float32)
    pf = partitions.astype(np.float32)
    for p in range(NPAIR):
        mmat[:, 2 * p] = pf[p]
        mmat[:, 2 * p + 1] = np.float32(1.0) - pf[p]
        invc8[2 * p] = np.float32(1.0) / pf[p].sum(dtype=np.float32)
        invc8[2 * p + 1] = np.float32(1.0) / (np.float32(1.0) - pf[p]).sum(
            dtype=np.float32)
    invc = invc8.reshape(J, 1).copy()
    memory = np.concatenate([state, state_memory[state.shape[0]:]], axis=0)

    def _relayout(arrT, f):
        # [D, f] row-major -> [128, NDC*f]: row p holds chunks k at cols k*f
        return np.ascontiguousarray(
            arrT.reshape(NDC, 128, f).transpose(1, 0, 2).reshape(128, NDC * f))

    mmat = _relayout(mmat, J)
    memt = _relayout(np.ascontiguousarray(memory.T), MEM)
    sampt = _relayout(np.ascontiguousarray(memory[sample_idx].T), SN)

    in_maps = []
    for c in range(N_CORES):
        htc = np.ascontiguousarray(state_history[c * TL:(c + 1) * TL, :].T)
        in_maps.append({"ht": htc, "mmat": mmat, "invc": invc,
                        "memt": memt, "sampt": sampt})

    nc = _get_nc(debug)
    res = run_bass_kernel_spmd(nc, in_maps, list(range(N_CORES)),
                               trace=trace)
    LAST_RESULTS = res
    return np.asarray(res.results[0]["out"], np.float32)


# revision 16
# speedup vs baseline: 1.4344x; 1.0306x over previous
"""Trainium2 Bass kernel for nn_ConsciousnessMonitor (histogram_binning).

kernel(**inputs) takes FULL unsharded numpy inputs, returns the full (9,)
float32 output. Shards state_history along time across 8 NeuronCores:
masked means via PE matmul while streaming (ht chunks stationary, mask
columns moving, so S arrives time-major), min/max + joint-histogram MI
with two small AllReduces, differentiation branch replicated per core.

Self-contained: shapes/sharding hardcoded; reads no sibling files.
"""
import numpy as np

import concourse.bacc as bacc
import concourse.tile as tile
import concourse.mybir as mybir
from concourse.bass_utils import run_bass_kernel_spmd
from concourse.masks import make_identity

F32 = mybir.dt.float32
I32 = mybir.dt.int32
BF16 = mybir.dt.bfloat16
AX = mybir.AxisListType
OP = mybir.AluOpType
ACT = mybir.ActivationFunctionType

N_CORES = 8
T, D = 32768, 2048
TL = T // N_CORES          # 4096 time steps per core
NB = 10                    # histogram bins per axis
NPAIR = 4                  # partitions (mask pairs)
J = 2 * NPAIR              # 8 masked-mean columns
NDC = D // 128             # 16 contraction chunks
NCH = TL // 128            # 32 time chunks of 128 (PSUM cols / binning)
MEM = 100
SN = 10

LN_T = float(np.log(np.float32(T)))
INV_T = 1.0 / T
EPS_N = T * 1e-10          # joint-count epsilon under common denominator
EPS_RC = float(T) * T * 1e-10  # outer-product epsilon likewise

_CACHE = {}
LAST_RESULTS = None


def _build(debug=False, variant="main"):
    sim1 = variant.startswith("sim1")
    nc = bacc.Bacc("TRN2", target_bir_lowering=False, debug=False,
                   num_devices=1 if sim1 else N_CORES)
    ht = nc.dram_tensor("ht", [D, TL], F32, kind="ExternalInput").ap()
    mmat = nc.dram_tensor("mmat", [128, NDC * J], F32,
                          kind="ExternalInput").ap()
    invc = nc.dram_tensor("invc", [J, 1], F32, kind="ExternalInput").ap()
    memt = nc.dram_tensor("memt", [128, NDC * MEM], F32,
                          kind="ExternalInput").ap()
    sampt = nc.dram_tensor("sampt", [128, NDC * SN], F32,
                           kind="ExternalInput").ap()
    out = nc.dram_tensor("out", [9], F32, kind="ExternalOutput").ap()
    if debug:
        dbg_st = nc.dram_tensor("dbg_st", [128, J], F32,
                                kind="ExternalOutput").ap()
        dbg_gmm = nc.dram_tensor("dbg_gmm", [J, 2], F32,
                                 kind="ExternalOutput").ap()
        dbg_s1b1 = nc.dram_tensor("dbg_s1b1", [J, 2], F32,
                                  kind="ExternalOutput").ap()
        dbg_bin = nc.dram_tensor("dbg_bin", [128, 16], I32,
                                 kind="ExternalOutput").ap()
        dbg_gj = nc.dram_tensor("dbg_gj", [NB, NPAIR * NB], F32,
                                kind="ExternalOutput").ap()

    rg = [list(range(N_CORES))]

    with tile.TileContext(nc) as tc:
        with tc.tile_pool(name="consts", bufs=1) as consts, \
             tc.tile_pool(name="sb", bufs=1) as sb, \
             tc.tile_pool(name="htp", bufs=2) as htp, \
             tc.tile_pool(name="psA", bufs=1, space="PSUM") as psA_pool, \
             tc.tile_pool(name="psJ", bufs=1, space="PSUM") as psJ_pool, \
             tc.tile_pool(name="misc", bufs=3, space="PSUM") as misc, \
             tc.tile_pool(name="dram", bufs=1, space="DRAM") as dram:

            # ---- constants / small inputs ----
            ident10 = consts.tile([NB, NB], F32, tag="id10")
            make_identity(nc, ident10[:])
            ident128 = consts.tile([128, 128], F32, tag="id128")
            make_identity(nc, ident128[:])
            ones128 = consts.tile([128, 1], F32, tag="o128")
            nc.gpsimd.memset(ones128[:], 1.0)
            ones10 = consts.tile([NB, 1], F32, tag="o10")
            nc.gpsimd.memset(ones10[:], 1.0)
            ones1_10 = consts.tile([1, NB], F32, tag="o110")
            nc.gpsimd.memset(ones1_10[:], 1.0)
            ones1_128 = consts.tile([1, 128], F32, tag="o1128")
            nc.gpsimd.memset(ones1_128[:], 1.0)

            # preload the Ln activation table early (Sqrt/Tanh load via the
            # differentiation branch) so no table load lands on the tail
            lnwarm = sb.tile([1, 1], F32, tag="lnwarm")
            nc.scalar.activation(lnwarm[:], ones10[0:1, :1], ACT.Ln)
            cepsrc = consts.tile([NB, 1], F32, tag="cepsrc")
            nc.gpsimd.memset(cepsrc[:], EPS_RC)
            cepsn = consts.tile([NB, 1], F32, tag="cepsn")
            nc.gpsimd.memset(cepsn[:], EPS_N)

            htt0 = htp.tile([128, TL], F32, tag="htt", name="htt")
            nc.sync.dma_start(out=htt0[:], in_=ht[0:128, :])
            m_sb = consts.tile([128, NDC * J], F32, tag="msb")
            nc.sync.dma_start(out=m_sb[:], in_=mmat[:])
            invc_sb = consts.tile([J, 1], F32, tag="invc")
            nc.gpsimd.dma_start(out=invc_sb[:], in_=invc[:])
            mem_sb = consts.tile([128, NDC * MEM], F32, tag="memsb")
            nc.gpsimd.dma_start(out=mem_sb[:], in_=memt[:])
            samp_sb = consts.tile([128, NDC * SN], F32, tag="sampsb")
            nc.gpsimd.dma_start(out=samp_sb[:], in_=sampt[:])

            # ---- differentiation branch (all early; overlaps stream) ----
            psG = misc.tile([SN, SN], F32, tag="m")
            for k in range(NDC):
                nc.tensor.matmul(psG[:], samp_sb[:, k * SN:(k + 1) * SN],
                                 samp_sb[:, k * SN:(k + 1) * SN],
                                 start=(k == 0), stop=(k == NDC - 1))
            sqs = sb.tile([128, NDC * SN], F32, tag="sqs")
            nc.vector.tensor_tensor(sqs[:], samp_sb[:], samp_sb[:], OP.mult)
            psr = misc.tile([SN, 1], F32, tag="m")
            for k in range(NDC):
                nc.tensor.matmul(psr[:], sqs[:, k * SN:(k + 1) * SN],
                                 ones128[:], start=(k == 0),
                                 stop=(k == NDC - 1))
            g_sb = sb.tile([SN, SN], F32, tag="gsb")
            nc.scalar.copy(g_sb[:], psG[:])
            r_sb = sb.tile([SN, 1], F32, tag="rsb")
            nc.scalar.copy(r_sb[:], psr[:])

            # variance branch (DVE; early)
            mem3 = mem_sb[:].rearrange("p (k f) -> p k f", f=MEM)
            mean16 = sb.tile([128, NDC], F32, tag="mean16")
            nc.vector.tensor_reduce(mean16[:], mem3, AX.X, OP.add)
            nc.vector.tensor_scalar(mean16[:], mean16[:], 1.0 / MEM, None,
                                    OP.mult)
            cent = sb.tile([128, NDC * MEM], F32, tag="cent")
            nc.vector.tensor_tensor(
                cent[:].rearrange("p (k f) -> p k f", f=MEM), mem3,
                mean16[:, :, None].broadcast_to([128, NDC, MEM]), OP.subtract)
            nc.vector.tensor_tensor(cent[:], cent[:], cent[:], OP.mult)
            var16 = sb.tile([128, NDC], F32, tag="var16")
            nc.vector.tensor_reduce(
                var16[:], cent[:].rearrange("p (k f) -> p k f", f=MEM),
                AX.X, OP.add)
            nc.vector.tensor_scalar(var16[:], var16[:], 1.0 / (MEM - 1), None,
                                    OP.mult)
            redv = sb.tile([128, 1], F32, tag="redv")
            nc.vector.tensor_reduce(redv[:], var16[:], AX.X, OP.add)
            v2 = sb.tile([128, NDC], F32, tag="v2")
            nc.vector.tensor_tensor(v2[:], var16[:], var16[:], OP.mult)
            redv2 = sb.tile([128, 1], F32, tag="redv2")
            nc.vector.tensor_reduce(redv2[:], v2[:], AX.X, OP.add)
            pstv = misc.tile([1, 1], F32, tag="m")
            nc.tensor.matmul(pstv[:], redv[:], ones128[:], start=True,
                             stop=True)
            tv_sb = sb.tile([1, 1], F32, tag="tvsb")
            nc.scalar.copy(tv_sb[:], pstv[:])
            pss2 = misc.tile([1, 1], F32, tag="m")
            nc.tensor.matmul(pss2[:], redv2[:], ones128[:], start=True,
                             stop=True)
            s2_sb = sb.tile([1, 1], F32, tag="s2sb")
            nc.scalar.copy(s2_sb[:], pss2[:])

            tvsq = sb.tile([1, 1], F32, tag="tvsq")
            nc.vector.tensor_tensor(tvsq[:], tv_sb[:], tv_sb[:], OP.mult)
            dden = sb.tile([1, 1], F32, tag="dden")
            nc.vector.scalar_tensor_tensor(dden[:], tvsq[:], 1e-6, s2_sb[:],
                                           OP.mult, OP.add)
            rdden = sb.tile([1, 1], F32, tag="rdden")
            nc.vector.reciprocal(rdden[:], dden[:])
            eff_sb = sb.tile([1, 1], F32, tag="effsb")
            nc.vector.tensor_tensor(eff_sb[:], tvsq[:], rdden[:], OP.mult)

            # cdist tail: d2 = r_i + r_j - 2G
            rrow_ps = misc.tile([1, SN], F32, tag="m")
            nc.tensor.transpose(rrow_ps[:], r_sb[:], ident10[:])
            rrow = sb.tile([1, SN], F32, tag="rrow")
            nc.scalar.copy(rrow[:], rrow_ps[:])
            rB = misc.tile([SN, SN], F32, tag="m")
            nc.tensor.matmul(rB[:], ones1_10[:], rrow[:], start=True,
                             stop=True)
            d2 = sb.tile([SN, SN], F32, tag="d2")
            nc.vector.scalar_tensor_tensor(d2[:], g_sb[:], -2.0, rB[:],
                                           OP.mult, OP.add)
            nc.vector.tensor_scalar(d2[:], d2[:], r_sb[:], 0.0, OP.add,
                                    OP.max)
            dst = sb.tile([SN, SN], F32, tag="dst")
            nc.scalar.activation(dst[:], d2[:], ACT.Sqrt)
            dsum = sb.tile([SN, 1], F32, tag="dsum")
            nc.vector.tensor_reduce(dsum[:], dst[:], AX.X, OP.add)
            psD = misc.tile([1, 1], F32, tag="m")
            nc.tensor.matmul(psD[:], dsum[:], ones10[:], start=True, stop=True)
            avg_sb = sb.tile([1, 1], F32, tag="avgsb")
            nc.vector.tensor_scalar(avg_sb[:], psD[:],
                                    float(1.0 / (SN * (SN - 1) + 1e-6)), None,
                                    OP.mult)
            sqtv = sb.tile([1, 1], F32, tag="sqtv")
            nc.scalar.activation(sqtv[:], tv_sb[:], ACT.Sqrt)
            diff_sb = sb.tile([1, 1], F32, tag="diffsb")
            nc.vector.tensor_tensor(diff_sb[:], sqtv[:], avg_sb[:], OP.mult)
            tanhd = sb.tile([1, 1], F32, tag="tanhd")
            nc.scalar.activation(tanhd[:], diff_sb[:], ACT.Tanh)
            outrow = sb.tile([1, 9], F32, tag="outrow")
            nc.vector.tensor_copy(outrow[:, 1:2], diff_sb[:])
            nc.vector.tensor_copy(outrow[:, 2:3], eff_sb[:])
            nc.vector.tensor_copy(outrow[:, 3:4], tv_sb[:])

            # ---- stage A: stream HT; ht chunks stationary, masks moving ----
            # psAll[:, c*J+j] accumulates S.T[t, j] for t-chunk c: 128 t rows
            # on partitions, all 32 chunks x 8 series in half a PSUM bank.
            psAll = psA_pool.tile([128, NCH * J], F32, tag="sacc")
            for dk in range(NDC):
                if dk == 0:
                    htt = htt0
                elif dk == NDC - 1:
                    # halves on both queues so the tail after the last byte
                    # only covers 16 matmuls
                    htt = htp.tile([128, TL], F32, tag="htt", name="htt")
                    half = TL // 2
                    nc.sync.dma_start(out=htt[:, 0:half],
                                      in_=ht[dk * 128:(dk + 1) * 128, 0:half])
                    nc.gpsimd.dma_start(
                        out=htt[:, half:TL],
                        in_=ht[dk * 128:(dk + 1) * 128, half:TL])
                else:
                    htt = htp.tile([128, TL], F32, tag="htt", name="htt")
                    q = nc.sync if (dk % 2 == 0) else nc.gpsimd
                    q.dma_start(out=htt[:],
                                in_=ht[dk * 128:(dk + 1) * 128, :])
                for c in range(NCH):
                    # start zeroes the whole 2KB zero-region (bank), so only
                    # the very first matmul in the bank may carry start=True
                    nc.tensor.matmul(psAll[:, c * J:(c + 1) * J],
                                     htt[:, c * 128:(c + 1) * 128],
                                     m_sb[:, dk * J:(dk + 1) * J],
                                     start=(dk == 0 and c == 0),
                                     stop=(dk == NDC - 1 and c == NCH - 1),
                                     skip_group_check=True)

            # ---- stage B: raw min/max per series, scale, AllReduce(max) ----
            ps3 = psAll[:].rearrange("p (c j) -> p j c", j=J)
            mx8 = sb.tile([128, J], F32, tag="mx8")
            mn8 = sb.tile([128, J], F32, tag="mn8")
            nc.vector.tensor_reduce(mx8[:], ps3, AX.X, OP.max)
            nc.vector.tensor_reduce(mn8[:], ps3, AX.X, OP.min)
            psmx = misc.tile([J, 128], F32, tag="m", name="psmx")
            nc.tensor.transpose(psmx[:], mx8[:], ident128[:])
            psmn = misc.tile([J, 128], F32, tag="m", name="psmn")
            nc.tensor.transpose(psmn[:], mn8[:], ident128[:])
            minmax = sb.tile([J, 2], F32, tag="minmax")
            # scaled max = invc * raw max ; second col = -(invc * raw min)
            tmx = sb.tile([J, 1], F32, tag="tmx")
            nc.vector.tensor_reduce(tmx[:], psmx[:], AX.X, OP.max)
            tmn = sb.tile([J, 1], F32, tag="tmn")
            nc.vector.tensor_reduce(tmn[:], psmn[:], AX.X, OP.min)
            nc.vector.tensor_scalar(minmax[:, 0:1], tmx[:], invc_sb[:], None,
                                    OP.mult)
            nc.gpsimd.tensor_scalar(minmax[:, 1:2], tmn[:], invc_sb[:], -1.0,
                                    OP.mult, OP.mult)
            cbA = dram.tile([J, 2], F32, tag="cba")
            cbB = dram.tile([J, 2], F32, tag="cbb")
            nc.sync.dma_start(out=cbA[:], in_=minmax[:])
            if sim1:
                nc.sync.dma_start(out=cbB[:], in_=cbA[:])
            else:
                nc.gpsimd.collective_compute("AllReduce", OP.max,
                                             replica_groups=rg,
                                             ins=[cbA.opt()],
                                             outs=[cbB.opt()])
            gmm = sb.tile([J, 2], F32, tag="gmm")
            nc.sync.dma_start(out=gmm[:], in_=cbB[:])

            # s1 = 10/(max-min+1e-6); b1 = -min*s1 - 0.5 (RNE cast -> floor)
            # on raw S: s1eff = s1*invc (gmm holds scaled max / -scaled min)
            rng8 = sb.tile([J, 1], F32, tag="rng8")
            nc.vector.tensor_reduce(rng8[:], gmm[:], AX.X, OP.add)
            nc.vector.tensor_scalar(rng8[:], rng8[:], 1e-6, None, OP.add)
            rdd = sb.tile([J, 1], F32, tag="rdd")
            nc.vector.reciprocal(rdd[:], rng8[:])
            sbcol = sb.tile([J, 2], F32, tag="sbcol")
            # col0: s1eff = 10*invc/(range) ; col1: b1 = gmm[:,1]*s1 - 0.5
            s1 = sb.tile([J, 1], F32, tag="s1")
            nc.vector.tensor_scalar(s1[:], rdd[:], 10.0, None, OP.mult)
            nc.vector.tensor_scalar(sbcol[:, 0:1], s1[:], invc_sb[:], None,
                                    OP.mult)
            nc.vector.tensor_tensor(sbcol[:, 1:2], gmm[:, 1:2], s1[:],
                                    OP.mult)
            nc.vector.tensor_scalar(sbcol[:, 1:2], sbcol[:, 1:2], -0.5, None,
                                    OP.add)
            # broadcast [J,2] -> rows: transpose then K=1 matmul -> [128, J]
            psTs = misc.tile([1, J], F32, tag="m", name="psTs")
            nc.tensor.transpose(psTs[:], sbcol[:, 0:1], ident10[0:J, 0:J])
            psTb = misc.tile([1, J], F32, tag="m", name="psTb")
            nc.tensor.transpose(psTb[:], sbcol[:, 1:2], ident10[0:J, 0:J])
            rowsb = sb.tile([1, 2 * J], F32, tag="rowsb")
            nc.scalar.copy(rowsb[:, 0:J], psTs[:])
            nc.scalar.copy(rowsb[:, J:2 * J], psTb[:])
            psbc = misc.tile([128, 2 * J], F32, tag="m", name="psbc")
            nc.tensor.matmul(psbc[:, 0:J], ones1_128[:], rowsb[:, 0:J],
                             start=True, stop=False, skip_group_check=True)
            nc.tensor.matmul(psbc[:, J:2 * J], ones1_128[:],
                             rowsb[:, J:2 * J], start=False, stop=True,
                             skip_group_check=True)
            s1rep = sb.tile([128, 2 * J], F32, tag="s1rep")
            nc.scalar.copy(s1rep[:], psbc[:])

            # ---- stage C: affine + int-cast + clamp + one-hot + joints ----
            binf = sb.tile([128, NCH * J], F32, tag="binf")
            b3 = binf[:].rearrange("p (c j) -> p c j", j=J)
            nc.vector.tensor_tensor(
                b3, psAll[:].rearrange("p (c j) -> p c j", j=J),
                s1rep[:, None, 0:J].broadcast_to([128, NCH, J]), OP.mult)
            binint = sb.tile([128, NCH * J], I32, tag="binint")
            nc.vector.tensor_tensor(
                binint[:].rearrange("p (c j) -> p c j", j=J), b3,
                s1rep[:, None, J:2 * J].broadcast_to([128, NCH, J]),
                OP.add)
            nc.vector.tensor_scalar(binint[:], binint[:], 0, NB - 1, OP.max,
                                    OP.min)
            ohsb = sb.tile([128, NCH * J * NB], BF16, tag="ohsb")
            oh3 = ohsb[:].rearrange("p (c b) -> p c b", b=NB)
            for b in range(NB):
                eng = nc.vector if b % 2 == 0 else nc.gpsimd
                eng.tensor_scalar(oh3[:, :, b], binint[:], b, None,
                                  OP.is_equal)
            # joint histograms: all 4 pairs side by side in one PSUM bank
            psJt = psJ_pool.tile([NB, NPAIR * NB], F32, tag="pj")
            for c in range(NCH):
                for p in range(NPAIR):
                    xa = (c * J + 2 * p) * NB
                    ya = (c * J + 2 * p + 1) * NB
                    nc.tensor.matmul(psJt[:, p * NB:(p + 1) * NB],
                                     ohsb[:, xa:xa + NB],
                                     ohsb[:, ya:ya + NB],
                                     start=(c == 0 and p == 0),
                                     stop=(c == NCH - 1 and p == NPAIR - 1),
                                     skip_group_check=True)
            gjl = sb.tile([NB, NPAIR * NB], F32, tag="gjl")
            nc.scalar.copy(gjl[:], psJt[:])
            cbj = dram.tile([NB, NPAIR * NB], F32, tag="cbj")
            cbj2 = dram.tile([NB, NPAIR * NB], F32, tag="cbj2")
            nc.sync.dma_start(out=cbj[:], in_=gjl[:])
            if sim1:
                nc.sync.dma_start(out=cbj2[:], in_=cbj[:])
            else:
                nc.gpsimd.collective_compute("AllReduce", OP.add,
                                             replica_groups=rg,
                                             ins=[cbj.opt()],
                                             outs=[cbj2.opt()])
            gj = sb.tile([NB, NPAIR * NB], F32, tag="gj")
            nc.sync.dma_start(out=gj[:], in_=cbj2[:])

            # ---- stage D: batched MI over the 4 pairs ----
            # mi_p = (1/T) sum_ij n_ij*(ln(n_ij+EPS_N)+LN_T-ln(r_i*c_j+EPS_RC))
            gj3 = gj[:].rearrange("a (p b) -> a p b", b=NB)
            r4 = sb.tile([NB, NPAIR], F32, tag="r4")
            nc.vector.tensor_reduce(r4[:], gj3, AX.X, OP.add)
            psc1 = misc.tile([1, NPAIR * NB], F32, tag="m", name="psc1")
            nc.tensor.matmul(psc1[:], ones10[:], gj[:], start=True, stop=True)
            c1 = sb.tile([1, NPAIR * NB], F32, tag="c1")
            nc.scalar.copy(c1[:], psc1[:])
            pscB = misc.tile([NB, NPAIR * NB], F32, tag="m", name="pscB")
            nc.tensor.matmul(pscB[:], ones1_10[:], c1[:], start=True,
                             stop=True)
            rc = sb.tile([NB, NPAIR * NB], F32, tag="rc")
            nc.vector.tensor_tensor(
                rc[:].rearrange("a (p b) -> a p b", b=NB),
                pscB[:].rearrange("a (p b) -> a p b", b=NB),
                r4[:, :, None].broadcast_to([NB, NPAIR, NB]), OP.mult)
            lnrc = sb.tile([NB, NPAIR * NB], F32, tag="lnrc")
            nc.scalar.activation(lnrc[:], rc[:], ACT.Ln, bias=cepsrc[:])
            lnn = sb.tile([NB, NPAIR * NB], F32, tag="lnn")
            nc.scalar.activation(lnn[:], gj[:], ACT.Ln, bias=cepsn[:])
            lterm = sb.tile([NB, NPAIR * NB], F32, tag="lterm")
            nc.vector.scalar_tensor_tensor(lterm[:], lnn[:], LN_T, lnrc[:],
                                           OP.add, OP.subtract)
            nc.vector.tensor_tensor(lterm[:], gj[:], lterm[:], OP.mult)
            rsum = sb.tile([NB, NPAIR], F32, tag="rsum")
            nc.vector.tensor_reduce(
                rsum[:], lterm[:].rearrange("a (p b) -> a p b", b=NB),
                AX.X, OP.add)
            psmi = misc.tile([1, NPAIR], F32, tag="m", name="psmi")
            nc.tensor.matmul(psmi[:], ones10[:], rsum[:], start=True,
                             stop=True)
            nc.vector.tensor_scalar(outrow[:, 5:9], psmi[:], INV_T, 0.0,
                                    OP.mult, OP.max)
            nc.vector.tensor_reduce(outrow[:, 4:5], outrow[:, 5:9], AX.X,
                                    OP.min)
            nc.vector.tensor_tensor(outrow[:, 0:1], outrow[:, 4:5], tanhd[:],
                                    OP.add)
            nc.sync.dma_start(out=out[:], in_=outrow[:])
            if debug:
                nc.sync.dma_start(out=dbg_st[:], in_=binf[:, 0:J])
                nc.sync.dma_start(out=dbg_gmm[:], in_=gmm[:])
                nc.sync.dma_start(out=dbg_s1b1[:], in_=sbcol[:])
                nc.sync.dma_start(out=dbg_bin[:], in_=binint[:, 0:16])
                nc.sync.dma_start(out=dbg_gj[:], in_=gj[:])

    nc.compile()
    return nc


def _build_variant(name):
    return _build(variant=name)


def _get_nc(debug=False):
    key = ("ncd" if debug else "nc")
    if key not in _CACHE:
        _CACHE[key] = _build(debug)
    return _CACHE[key]


def kernel(state, state_memory, state_history, partitions, sample_idx,
           trace=False, debug=False):
    global LAST_RESULTS
    state = np.asarray(state, np.float32)
    state_memory = np.asarray(state_memory, np.float32)
    state_history = np.asarray(state_history, np.float32)
    partitions = np.asarray(partitions)
    sample_idx = np.asarray(sample_idx)

    mmat = np.empty((D, J), np.float32)
    invc8 = np.empty((J,), np.float32)
    pf = partitions.astype(np.float32)
    for p in range(NPAIR):
        mmat[:, 2 * p] = pf[p]
        mmat[:, 2 * p + 1] = np.float32(1.0) - pf[p]
        invc8[2 * p] = np.float32(1.0) / pf[p].sum(dtype=np.float32)
        invc8[2 * p + 1] = np.float32(1.0) / (np.float32(1.0) - pf[p]).sum(
            dtype=np.float32)
    invc = invc8.reshape(J, 1).copy()
    memory = np.concatenate([state, state_memory[state.shape[0]:]], axis=0)

    def _relayout(arrT, f):
        # [D, f] row-major -> [128, NDC*f]: row p holds chunks k at cols k*f
        return np.ascontiguousarray(
            arrT.reshape(NDC, 128, f).transpose(1, 0, 2).reshape(128, NDC * f))

    mmat = _relayout(mmat, J)
    memt = _relayout(np.ascontiguousarray(memory.T), MEM)
    sampt = _relayout(np.ascontiguousarray(memory[sample_idx].T), SN)

    in_maps = []
    for c in range(N_CORES):
        htc = np.ascontiguousarray(state_history[c * TL:(c + 1) * TL, :].T)
        in_maps.append({"ht": htc, "mmat": mmat, "invc": invc,
                        "memt": memt, "sampt": sampt})

    nc = _get_nc(debug)
    res = run_bass_kernel_spmd(nc, in_maps, list(range(N_CORES)),
                               trace=trace)
    LAST_RESULTS = res
    return np.asarray(res.results[0]["out"], np.float32)


# revision 18
# speedup vs baseline: 1.4526x; 1.0127x over previous
"""Trainium2 Bass kernel for nn_ConsciousnessMonitor (histogram_binning).

kernel(**inputs) takes FULL unsharded numpy inputs, returns the full (9,)
float32 output. Shards state_history along time across 8 NeuronCores:
masked means via PE matmul while streaming (ht chunks stationary, mask
columns moving, so S arrives time-major), min/max + joint-histogram MI
with two small AllReduces, differentiation branch replicated per core.

Self-contained: shapes/sharding hardcoded; reads no sibling files.
"""
import numpy as np

import concourse.bacc as bacc
import concourse.tile as tile
import concourse.mybir as mybir
from concourse.bass_utils import run_bass_kernel_spmd
from concourse.masks import make_identity

F32 = mybir.dt.float32
I32 = mybir.dt.int32
BF16 = mybir.dt.bfloat16
AX = mybir.AxisListType
OP = mybir.AluOpType
ACT = mybir.ActivationFunctionType

N_CORES = 8
T, D = 32768, 2048
TL = T // N_CORES          # 4096 time steps per core
NB = 10                    # histogram bins per axis
NPAIR = 4                  # partitions (mask pairs)
J = 2 * NPAIR              # 8 masked-mean columns
NDC = D // 128             # 16 contraction chunks
NCH = TL // 128            # 32 time chunks of 128 (PSUM cols / binning)
MEM = 100
SN = 10

LN_T = float(np.log(np.float32(T)))
INV_T = 1.0 / T
EPS_N = T * 1e-10          # joint-count epsilon under common denominator
EPS_RC = float(T) * T * 1e-10  # outer-product epsilon likewise

_CACHE = {}
LAST_RESULTS = None


def _build(debug=False, variant="main"):
    sim1 = variant.startswith("sim1")
    nc = bacc.Bacc("TRN2", target_bir_lowering=False, debug=False,
                   num_devices=1 if sim1 else N_CORES)
    ht = nc.dram_tensor("ht", [D, TL], F32, kind="ExternalInput").ap()
    mmat = nc.dram_tensor("mmat", [128, NDC * J], F32,
                          kind="ExternalInput").ap()
    invc = nc.dram_tensor("invc", [2 * J, 1], F32,
                      kind="ExternalInput").ap()
    memt = nc.dram_tensor("memt", [128, NDC * MEM], F32,
                          kind="ExternalInput").ap()
    sampt = nc.dram_tensor("sampt", [128, NDC * SN], F32,
                           kind="ExternalInput").ap()
    out = nc.dram_tensor("out", [9], F32, kind="ExternalOutput").ap()
    if debug:
        dbg_st = nc.dram_tensor("dbg_st", [128, J], F32,
                                kind="ExternalOutput").ap()
        dbg_gmm = nc.dram_tensor("dbg_gmm", [2 * J, 2], F32,
                                 kind="ExternalOutput").ap()
        dbg_s1b1 = nc.dram_tensor("dbg_s1b1", [1, 2 * J], F32,
                                  kind="ExternalOutput").ap()
        dbg_bin = nc.dram_tensor("dbg_bin", [128, 16], I32,
                                 kind="ExternalOutput").ap()
        dbg_gj = nc.dram_tensor("dbg_gj", [NB, NPAIR * NB], F32,
                                kind="ExternalOutput").ap()

    rg = [list(range(N_CORES))]

    with tile.TileContext(nc) as tc:
        with tc.tile_pool(name="consts", bufs=1) as consts, \
             tc.tile_pool(name="sb", bufs=1) as sb, \
             tc.tile_pool(name="htp", bufs=2) as htp, \
             tc.tile_pool(name="psA", bufs=1, space="PSUM") as psA_pool, \
             tc.tile_pool(name="psJ", bufs=1, space="PSUM") as psJ_pool, \
             tc.tile_pool(name="misc", bufs=3, space="PSUM") as misc, \
             tc.tile_pool(name="dram", bufs=1, space="DRAM") as dram:

            # ---- constants / small inputs ----
            ident10 = consts.tile([NB, NB], F32, tag="id10")
            make_identity(nc, ident10[:])
            ident128 = consts.tile([128, 128], F32, tag="id128")
            make_identity(nc, ident128[:])
            ones128 = consts.tile([128, 1], F32, tag="o128")
            nc.gpsimd.memset(ones128[:], 1.0)
            ones10 = consts.tile([NB, 1], F32, tag="o10")
            nc.gpsimd.memset(ones10[:], 1.0)
            ones1_10 = consts.tile([1, NB], F32, tag="o110")
            nc.gpsimd.memset(ones1_10[:], 1.0)
            ones1_128 = consts.tile([1, 128], F32, tag="o1128")
            nc.gpsimd.memset(ones1_128[:], 1.0)
            ones10x10 = consts.tile([NB, NB], F32, tag="o1010")
            nc.gpsimd.memset(ones10x10[:], 1.0)

            cepsrc = consts.tile([NB, 1], F32, tag="cepsrc")
            nc.gpsimd.memset(cepsrc[:], EPS_RC)
            cepsn = consts.tile([NB, 1], F32, tag="cepsn")
            nc.gpsimd.memset(cepsn[:], EPS_N)

            htt0 = htp.tile([128, TL], F32, tag="htt", name="htt")
            nc.sync.dma_start(out=htt0[:], in_=ht[0:128, :])
            m_sb = consts.tile([128, NDC * J], F32, tag="msb")
            nc.sync.dma_start(out=m_sb[:], in_=mmat[:])
            invc_sb = consts.tile([2 * J, 1], F32, tag="invc")
            nc.gpsimd.dma_start(out=invc_sb[:], in_=invc[:])
            mem_sb = consts.tile([128, NDC * MEM], F32, tag="memsb")
            nc.gpsimd.dma_start(out=mem_sb[:], in_=memt[:])
            samp_sb = consts.tile([128, NDC * SN], F32, tag="sampsb")
            nc.gpsimd.dma_start(out=samp_sb[:], in_=sampt[:])

            # ---- differentiation branch (all early; overlaps stream) ----
            psG = misc.tile([SN, SN], F32, tag="m")
            for k in range(NDC):
                nc.tensor.matmul(psG[:], samp_sb[:, k * SN:(k + 1) * SN],
                                 samp_sb[:, k * SN:(k + 1) * SN],
                                 start=(k == 0), stop=(k == NDC - 1))
            sqs = sb.tile([128, NDC * SN], F32, tag="sqs")
            nc.vector.tensor_tensor(sqs[:], samp_sb[:], samp_sb[:], OP.mult)
            psr = misc.tile([SN, 1], F32, tag="m")
            for k in range(NDC):
                nc.tensor.matmul(psr[:], sqs[:, k * SN:(k + 1) * SN],
                                 ones128[:], start=(k == 0),
                                 stop=(k == NDC - 1))
            g_sb = sb.tile([SN, SN], F32, tag="gsb")
            nc.scalar.copy(g_sb[:], psG[:])
            r_sb = sb.tile([SN, 1], F32, tag="rsb")
            nc.scalar.copy(r_sb[:], psr[:])

            # variance branch (DVE; early)
            mem3 = mem_sb[:].rearrange("p (k f) -> p k f", f=MEM)
            mean16 = sb.tile([128, NDC], F32, tag="mean16")
            nc.vector.tensor_reduce(mean16[:], mem3, AX.X, OP.add)
            nc.vector.tensor_scalar(mean16[:], mean16[:], 1.0 / MEM, None,
                                    OP.mult)
            cent = sb.tile([128, NDC * MEM], F32, tag="cent")
            nc.vector.tensor_tensor(
                cent[:].rearrange("p (k f) -> p k f", f=MEM), mem3,
                mean16[:, :, None].broadcast_to([128, NDC, MEM]), OP.subtract)
            nc.vector.tensor_tensor(cent[:], cent[:], cent[:], OP.mult)
            var16 = sb.tile([128, NDC], F32, tag="var16")
            nc.vector.tensor_reduce(
                var16[:], cent[:].rearrange("p (k f) -> p k f", f=MEM),
                AX.X, OP.add)
            nc.vector.tensor_scalar(var16[:], var16[:], 1.0 / (MEM - 1), None,
                                    OP.mult)
            redv = sb.tile([128, 1], F32, tag="redv")
            nc.vector.tensor_reduce(redv[:], var16[:], AX.X, OP.add)
            v2 = sb.tile([128, NDC], F32, tag="v2")
            nc.vector.tensor_tensor(v2[:], var16[:], var16[:], OP.mult)
            redv2 = sb.tile([128, 1], F32, tag="redv2")
            nc.vector.tensor_reduce(redv2[:], v2[:], AX.X, OP.add)
            pstv = misc.tile([1, 1], F32, tag="m")
            nc.tensor.matmul(pstv[:], redv[:], ones128[:], start=True,
                             stop=True)
            tv_sb = sb.tile([1, 1], F32, tag="tvsb")
            nc.scalar.copy(tv_sb[:], pstv[:])
            pss2 = misc.tile([1, 1], F32, tag="m")
            nc.tensor.matmul(pss2[:], redv2[:], ones128[:], start=True,
                             stop=True)
            s2_sb = sb.tile([1, 1], F32, tag="s2sb")
            nc.scalar.copy(s2_sb[:], pss2[:])

            tvsq = sb.tile([1, 1], F32, tag="tvsq")
            nc.vector.tensor_tensor(tvsq[:], tv_sb[:], tv_sb[:], OP.mult)
            dden = sb.tile([1, 1], F32, tag="dden")
            nc.vector.scalar_tensor_tensor(dden[:], tvsq[:], 1e-6, s2_sb[:],
                                           OP.mult, OP.add)
            rdden = sb.tile([1, 1], F32, tag="rdden")
            nc.vector.reciprocal(rdden[:], dden[:])
            eff_sb = sb.tile([1, 1], F32, tag="effsb")
            nc.vector.tensor_tensor(eff_sb[:], tvsq[:], rdden[:], OP.mult)

            # cdist tail: d2 = r_i + r_j - 2G
            rrow_ps = misc.tile([1, SN], F32, tag="m")
            nc.tensor.transpose(rrow_ps[:], r_sb[:], ident10[:])
            rrow = sb.tile([1, SN], F32, tag="rrow")
            nc.scalar.copy(rrow[:], rrow_ps[:])
            rB = misc.tile([SN, SN], F32, tag="m")
            nc.tensor.matmul(rB[:], ones1_10[:], rrow[:], start=True,
                             stop=True)
            d2 = sb.tile([SN, SN], F32, tag="d2")
            nc.vector.scalar_tensor_tensor(d2[:], g_sb[:], -2.0, rB[:],
                                           OP.mult, OP.add)
            nc.vector.tensor_scalar(d2[:], d2[:], r_sb[:], 0.0, OP.add,
                                    OP.max)
            dst = sb.tile([SN, SN], F32, tag="dst")
            nc.scalar.activation(dst[:], d2[:], ACT.Sqrt)
            dsum = sb.tile([SN, 1], F32, tag="dsum")
            nc.vector.tensor_reduce(dsum[:], dst[:], AX.X, OP.add)
            psD = misc.tile([1, 1], F32, tag="m")
            nc.tensor.matmul(psD[:], dsum[:], ones10[:], start=True, stop=True)
            avg_sb = sb.tile([1, 1], F32, tag="avgsb")
            nc.vector.tensor_scalar(avg_sb[:], psD[:],
                                    float(1.0 / (SN * (SN - 1) + 1e-6)), None,
                                    OP.mult)
            sqtv = sb.tile([1, 1], F32, tag="sqtv")
            nc.scalar.activation(sqtv[:], tv_sb[:], ACT.Sqrt)
            diff_sb = sb.tile([1, 1], F32, tag="diffsb")
            nc.vector.tensor_tensor(diff_sb[:], sqtv[:], avg_sb[:], OP.mult)
            tanhd = sb.tile([1, 1], F32, tag="tanhd")
            nc.scalar.activation(tanhd[:], diff_sb[:], ACT.Tanh)
            # load the Ln act table now (Act idle until the MI lns) so no
            # table switch lands on the tail critical path
            lnwarm = sb.tile([1, 1], F32, tag="lnwarm")
            nc.scalar.activation(lnwarm[:], ones10[0:1, :1], ACT.Ln)
            outrow = sb.tile([1, 9], F32, tag="outrow")
            nc.vector.tensor_copy(outrow[:, 1:2], diff_sb[:])
            nc.vector.tensor_copy(outrow[:, 2:3], eff_sb[:])
            nc.vector.tensor_copy(outrow[:, 3:4], tv_sb[:])

            # ---- stage A: stream HT; ht chunks stationary, masks moving ----
            # psAll[:, c*J+j] accumulates S.T[t, j] for t-chunk c: 128 t rows
            # on partitions, all 32 chunks x 8 series in half a PSUM bank.
            psAll = psA_pool.tile([128, NCH * J], F32, tag="sacc")
            for dk in range(NDC):
                if dk == 0:
                    htt = htt0
                elif dk == NDC - 1:
                    # halves on both queues so the tail after the last byte
                    # only covers 16 matmuls
                    htt = htp.tile([128, TL], F32, tag="htt", name="htt")
                    half = TL // 2
                    nc.sync.dma_start(out=htt[:, 0:half],
                                      in_=ht[dk * 128:(dk + 1) * 128, 0:half])
                    nc.gpsimd.dma_start(
                        out=htt[:, half:TL],
                        in_=ht[dk * 128:(dk + 1) * 128, half:TL])
                else:
                    htt = htp.tile([128, TL], F32, tag="htt", name="htt")
                    q = nc.sync if (dk % 2 == 0) else nc.gpsimd
                    q.dma_start(out=htt[:],
                                in_=ht[dk * 128:(dk + 1) * 128, :])
                for c in range(NCH):
                    # start zeroes the whole 2KB zero-region (bank), so only
                    # the very first matmul in the bank may carry start=True
                    nc.tensor.matmul(psAll[:, c * J:(c + 1) * J],
                                     htt[:, c * 128:(c + 1) * 128],
                                     m_sb[:, dk * J:(dk + 1) * J],
                                     start=(dk == 0 and c == 0),
                                     stop=(dk == NDC - 1 and c == NCH - 1),
                                     skip_group_check=True)

            # ---- stage B: raw min/max per series, scale, AllReduce(max) ----
            # mxmn cols 0:8 = max, cols 8:16 = -min (so one max-reduce after
            # transpose covers both); AR payload col1 carries invc (constant
            # across cores, so max is the identity on it)
            ps3 = psAll[:].rearrange("p (c j) -> p j c", j=J)
            mxmn = sb.tile([128, 2 * J], F32, tag="mxmn")
            nc.vector.tensor_reduce(mxmn[:, 0:J], ps3, AX.X, OP.max)
            nc.vector.tensor_reduce(mxmn[:, J:2 * J], ps3, AX.X, OP.min)
            nc.vector.tensor_scalar(mxmn[:, J:2 * J], mxmn[:, J:2 * J], -1.0,
                                    None, OP.mult)
            psT = misc.tile([2 * J, 128], F32, tag="m", name="psT")
            nc.tensor.transpose(psT[:], mxmn[:], ident128[:])
            minmax = sb.tile([2 * J, 2], F32, tag="minmax")
            nc.vector.tensor_copy(minmax[:, 1:2], invc_sb[:])
            tmx = sb.tile([2 * J, 1], F32, tag="tmx")
            nc.vector.tensor_reduce(tmx[:], psT[:], AX.X, OP.max)
            nc.vector.tensor_scalar(minmax[:, 0:1], tmx[:], invc_sb[:], None,
                                    OP.mult)
            cbA = dram.tile([2 * J, 2], F32, tag="cba")
            cbB = dram.tile([2 * J, 2], F32, tag="cbb")
            nc.sync.dma_start(out=cbA[:], in_=minmax[:])
            if sim1:
                nc.sync.dma_start(out=cbB[:], in_=cbA[:])
            else:
                nc.gpsimd.collective_compute("AllReduce", OP.max,
                                             replica_groups=rg,
                                             ins=[cbA.opt()],
                                             outs=[cbB.opt()])
            # read back replicated on every partition: grow[p, 2r+c]=cbB[r,c]
            grow = sb.tile([128, 4 * J], F32, tag="grow")
            nc.sync.dma_start(
                out=grow[:],
                in_=cbB[:].rearrange("r c -> (r c)")[None, :]
                .broadcast_to([128, 4 * J]))
            # row-wise: rng = max+(-min); s1 = 10/(rng+1e-6);
            # s1eff = s1*invc; b1 = (-min)*s1 - 0.5
            gmax = grow[:, 0:2 * J:2]
            gnmn = grow[:, 2 * J:4 * J:2]
            ginv = grow[:, 1:2 * J:2]
            s1row = sb.tile([128, J], F32, tag="s1row")
            nc.vector.tensor_tensor(s1row[:], gmax, gnmn, OP.add)
            nc.vector.tensor_scalar(s1row[:], s1row[:], 1e-6, None, OP.add)
            nc.vector.reciprocal(s1row[:], s1row[:])
            nc.vector.tensor_scalar(s1row[:], s1row[:], 10.0, None, OP.mult)
            b1row = sb.tile([128, J], F32, tag="b1row")
            nc.vector.tensor_tensor(b1row[:], gnmn, s1row[:], OP.mult)
            nc.vector.tensor_scalar(b1row[:], b1row[:], -0.5, None, OP.add)
            s1eff = sb.tile([128, J], F32, tag="s1eff")
            nc.vector.tensor_tensor(s1eff[:], s1row[:], ginv, OP.mult)

            # ---- stage C: affine + int-cast + clamp + one-hot + joints ----
            binf = sb.tile([128, NCH * J], F32, tag="binf")
            b3 = binf[:].rearrange("p (c j) -> p c j", j=J)
            nc.vector.tensor_tensor(
                b3, psAll[:].rearrange("p (c j) -> p c j", j=J),
                s1eff[:, None, :].broadcast_to([128, NCH, J]), OP.mult)
            binint = sb.tile([128, NCH * J], I32, tag="binint")
            nc.vector.tensor_tensor(
                binint[:].rearrange("p (c j) -> p c j", j=J), b3,
                b1row[:, None, :].broadcast_to([128, NCH, J]),
                OP.add)
            nc.vector.tensor_scalar(binint[:], binint[:], 0, NB - 1, OP.max,
                                    OP.min)
            ohsb = sb.tile([128, NCH * J * NB], BF16, tag="ohsb")
            oh3 = ohsb[:].rearrange("p (c b) -> p c b", b=NB)
            for b in range(NB):
                eng = nc.vector if b % 2 == 0 else nc.gpsimd
                eng.tensor_scalar(oh3[:, :, b], binint[:], b, None,
                                  OP.is_equal)
            # joint histograms: all 4 pairs side by side in one PSUM bank
            psJt = psJ_pool.tile([NB, NPAIR * NB], F32, tag="pj")
            for c in range(NCH):
                for p in range(NPAIR):
                    xa = (c * J + 2 * p) * NB
                    ya = (c * J + 2 * p + 1) * NB
                    nc.tensor.matmul(psJt[:, p * NB:(p + 1) * NB],
                                     ohsb[:, xa:xa + NB],
                                     ohsb[:, ya:ya + NB],
                                     start=(c == 0 and p == 0),
                                     stop=(c == NCH - 1 and p == NPAIR - 1),
                                     skip_group_check=True)
            gjl = sb.tile([NB, NPAIR * NB], F32, tag="gjl")
            nc.vector.tensor_copy(gjl[:], psJt[:])
            cbj = dram.tile([NB, NPAIR * NB], F32, tag="cbj")
            cbj2 = dram.tile([NB, NPAIR * NB], F32, tag="cbj2")
            nc.sync.dma_start(out=cbj[:], in_=gjl[:])
            if sim1:
                nc.sync.dma_start(out=cbj2[:], in_=cbj[:])
            else:
                nc.gpsimd.collective_compute("AllReduce", OP.add,
                                             replica_groups=rg,
                                             ins=[cbj.opt()],
                                             outs=[cbj2.opt()])
            gj = sb.tile([NB, NPAIR * NB], F32, tag="gj")
            nc.sync.dma_start(out=gj[:], in_=cbj2[:])

            # ---- stage D: batched MI over the 4 pairs ----
            # mi_p = (1/T) sum_ij n_ij*(ln(n_ij+EPS_N)+LN_T-ln(r_i*c_j+EPS_RC))
            gj3 = gj[:].rearrange("a (p b) -> a p b", b=NB)
            r4 = sb.tile([NB, NPAIR], F32, tag="r4")
            nc.vector.tensor_reduce(r4[:], gj3, AX.X, OP.add)
            pscB = misc.tile([NB, NPAIR * NB], F32, tag="m", name="pscB")
            nc.tensor.matmul(pscB[:], ones10x10[:], gj[:], start=True,
                             stop=True)
            rc = sb.tile([NB, NPAIR * NB], F32, tag="rc")
            nc.vector.tensor_tensor(
                rc[:].rearrange("a (p b) -> a p b", b=NB),
                pscB[:].rearrange("a (p b) -> a p b", b=NB),
                r4[:, :, None].broadcast_to([NB, NPAIR, NB]), OP.mult)
            lnrc = sb.tile([NB, NPAIR * NB], F32, tag="lnrc")
            nc.scalar.activation(lnrc[:], rc[:], ACT.Ln, bias=cepsrc[:])
            lnn = sb.tile([NB, NPAIR * NB], F32, tag="lnn")
            nc.scalar.activation(lnn[:], gj[:], ACT.Ln, bias=cepsn[:])
            lterm = sb.tile([NB, NPAIR * NB], F32, tag="lterm")
            nc.vector.scalar_tensor_tensor(lterm[:], lnn[:], LN_T, lnrc[:],
                                           OP.add, OP.subtract)
            nc.vector.tensor_tensor(lterm[:], gj[:], lterm[:], OP.mult)
            rsum = sb.tile([NB, NPAIR], F32, tag="rsum")
            nc.vector.tensor_reduce(
                rsum[:], lterm[:].rearrange("a (p b) -> a p b", b=NB),
                AX.X, OP.add)
            psmi = misc.tile([1, NPAIR], F32, tag="m", name="psmi")
            nc.tensor.matmul(psmi[:], ones10[:], rsum[:], start=True,
                             stop=True)
            nc.vector.tensor_scalar(outrow[:, 5:9], psmi[:], INV_T, 0.0,
                                    OP.mult, OP.max)
            nc.vector.tensor_reduce(outrow[:, 4:5], outrow[:, 5:9], AX.X,
                                    OP.min)
            nc.vector.tensor_tensor(outrow[:, 0:1], outrow[:, 4:5], tanhd[:],
                                    OP.add)
            nc.sync.dma_start(out=out[:], in_=outrow[:])
            if debug:
                nc.sync.dma_start(out=dbg_st[:], in_=binf[:, 0:J])
                nc.sync.dma_start(out=dbg_gmm[:], in_=minmax[:])
                nc.sync.dma_start(out=dbg_s1b1[:, 0:J], in_=s1eff[0:1, :])
                nc.sync.dma_start(out=dbg_bin[:], in_=binint[:, 0:16])
                nc.sync.dma_start(out=dbg_gj[:], in_=gj[:])

    nc.compile()
    return nc


def _build_variant(name):
    return _build(variant=name)


def _get_nc(debug=False):
    key = ("ncd" if debug else "nc")
    if key not in _CACHE:
        _CACHE[key] = _build(debug)
    return _CACHE[key]


def kernel(state, state_memory, state_history, partitions, sample_idx,
           trace=False, debug=False):
    global LAST_RESULTS
    state = np.asarray(state, np.float32)
    state_memory = np.asarray(state_memory, np.float32)
    state_history = np.asarray(state_history, np.float32)
    partitions = np.asarray(partitions)
    sample_idx = np.asarray(sample_idx)

    mmat = np.empty((D, J), np.float32)
    invc8 = np.empty((J,), np.float32)
    pf = partitions.astype(np.float32)
    for p in range(NPAIR):
        mmat[:, 2 * p] = pf[p]
        mmat[:, 2 * p + 1] = np.float32(1.0) - pf[p]
        invc8[2 * p] = np.float32(1.0) / pf[p].sum(dtype=np.float32)
        invc8[2 * p + 1] = np.float32(1.0) / (np.float32(1.0) - pf[p]).sum(
            dtype=np.float32)
    invc = np.tile(invc8, 2).reshape(2 * J, 1).copy()
    memory = np.concatenate([state, state_memory[state.shape[0]:]], axis=0)

    def _relayout(arrT, f):
        # [D, f] row-major -> [128, NDC*f]: row p holds chunks k at cols k*f
        return np.ascontiguousarray(
            arrT.reshape(NDC, 128, f).transpose(1, 0, 2).reshape(128, NDC * f))

    mmat = _relayout(mmat, J)
    memt = _relayout(np.ascontiguousarray(memory.T), MEM)
    sampt = _relayout(np.ascontiguousarray(memory[sample_idx].T), SN)

    in_maps = []
    for c in range(N_CORES):
        htc = np.ascontiguousarray(state_history[c * TL:(c + 1) * TL, :].T)
        in_maps.append({"ht": htc, "mmat": mmat, "invc": invc,
                        "memt": memt, "sampt": sampt})

    nc = _get_nc(debug)
    res = run_bass_kernel_spmd(nc, in_maps, list(range(N_CORES)),
                               trace=trace)
    LAST_RESULTS = res
    return np.asarray(res.results[0]["out"], np.float32)


# revision 20
# speedup vs baseline: 1.4724x; 1.0136x over previous
"""Trainium2 Bass kernel for nn_ConsciousnessMonitor (histogram_binning).

kernel(**inputs) takes FULL unsharded numpy inputs, returns the full (9,)
float32 output. Shards state_history along time across 8 NeuronCores:
masked means via PE matmul while streaming (ht chunks stationary, mask
columns moving, so S arrives time-major), min/max + joint-histogram MI
with two small AllReduces, differentiation branch replicated per core.

Self-contained: shapes/sharding hardcoded; reads no sibling files.
"""
import numpy as np

import concourse.bacc as bacc
import concourse.tile as tile
import concourse.mybir as mybir
from concourse.bass_utils import run_bass_kernel_spmd
from concourse.masks import make_identity

F32 = mybir.dt.float32
I32 = mybir.dt.int32
BF16 = mybir.dt.bfloat16
AX = mybir.AxisListType
OP = mybir.AluOpType
ACT = mybir.ActivationFunctionType

N_CORES = 8
T, D = 32768, 2048
TL = T // N_CORES          # 4096 time steps per core
NB = 10                    # histogram bins per axis
NPAIR = 4                  # partitions (mask pairs)
J = 2 * NPAIR              # 8 masked-mean columns
NDC = D // 128             # 16 contraction chunks
NCH = TL // 128            # 32 time chunks of 128 (PSUM cols / binning)
MEM = 100
SN = 10

LN_T = float(np.log(np.float32(T)))
INV_T = 1.0 / T
EPS_N = T * 1e-10          # joint-count epsilon under common denominator
EPS_RC = float(T) * T * 1e-10  # outer-product epsilon likewise

_CACHE = {}
LAST_RESULTS = None


def _build(debug=False, variant="main"):
    sim1 = variant.startswith("sim1")
    nc = bacc.Bacc("TRN2", target_bir_lowering=False, debug=False,
                   num_devices=1 if sim1 else N_CORES)
    ht = nc.dram_tensor("ht", [D, TL], F32, kind="ExternalInput").ap()
    mmat = nc.dram_tensor("mmat", [128, NDC * J], F32,
                          kind="ExternalInput").ap()
    invc = nc.dram_tensor("invc", [2 * J, 2], F32,
                      kind="ExternalInput").ap()
    memt = nc.dram_tensor("memt", [128, NDC * MEM], F32,
                          kind="ExternalInput").ap()
    sampt = nc.dram_tensor("sampt", [128, NDC * SN], F32,
                           kind="ExternalInput").ap()
    out = nc.dram_tensor("out", [9], F32, kind="ExternalOutput").ap()
    if debug:
        dbg_st = nc.dram_tensor("dbg_st", [128, J], F32,
                                kind="ExternalOutput").ap()
        dbg_gmm = nc.dram_tensor("dbg_gmm", [2 * J, 2], F32,
                                 kind="ExternalOutput").ap()
        dbg_s1b1 = nc.dram_tensor("dbg_s1b1", [1, 2 * J], F32,
                                  kind="ExternalOutput").ap()
        dbg_bin = nc.dram_tensor("dbg_bin", [128, 16], I32,
                                 kind="ExternalOutput").ap()
        dbg_gj = nc.dram_tensor("dbg_gj", [NB, NPAIR * NB], F32,
                                kind="ExternalOutput").ap()

    rg = [list(range(N_CORES))]

    with tile.TileContext(nc) as tc:
        with tc.tile_pool(name="consts", bufs=1) as consts, \
             tc.tile_pool(name="sb", bufs=1) as sb, \
             tc.tile_pool(name="htp", bufs=2) as htp, \
             tc.tile_pool(name="psA", bufs=1, space="PSUM") as psA_pool, \
             tc.tile_pool(name="psJ", bufs=1, space="PSUM") as psJ_pool, \
             tc.tile_pool(name="misc", bufs=3, space="PSUM") as misc, \
             tc.tile_pool(name="dram", bufs=1, space="DRAM") as dram:

            # ---- constants / small inputs ----
            ident10 = consts.tile([NB, NB], F32, tag="id10")
            make_identity(nc, ident10[:])
            ident128 = consts.tile([128, 128], F32, tag="id128")
            make_identity(nc, ident128[:])
            ones128 = consts.tile([128, 1], F32, tag="o128")
            nc.gpsimd.memset(ones128[:], 1.0)
            ones10 = consts.tile([NB, 1], F32, tag="o10")
            nc.gpsimd.memset(ones10[:], 1.0)
            ones1_10 = consts.tile([1, NB], F32, tag="o110")
            nc.gpsimd.memset(ones1_10[:], 1.0)
            ones1_128 = consts.tile([1, 128], F32, tag="o1128")
            nc.gpsimd.memset(ones1_128[:], 1.0)
            ones10x10 = consts.tile([NB, NB], F32, tag="o1010")
            nc.gpsimd.memset(ones10x10[:], 1.0)

            cepsrc = consts.tile([NB, 1], F32, tag="cepsrc")
            nc.gpsimd.memset(cepsrc[:], EPS_RC)
            cepsn = consts.tile([NB, 1], F32, tag="cepsn")
            nc.gpsimd.memset(cepsn[:], EPS_N)

            htt0 = htp.tile([128, TL], F32, tag="htt", name="htt")
            nc.sync.dma_start(out=htt0[:], in_=ht[0:128, :])
            m_sb = consts.tile([128, NDC * J], F32, tag="msb")
            nc.sync.dma_start(out=m_sb[:], in_=mmat[:])
            invc_sb = consts.tile([2 * J, 2], F32, tag="invc")
            nc.gpsimd.dma_start(out=invc_sb[:], in_=invc[:])
            mem_sb = consts.tile([128, NDC * MEM], F32, tag="memsb")
            nc.gpsimd.dma_start(out=mem_sb[:], in_=memt[:])
            samp_sb = consts.tile([128, NDC * SN], F32, tag="sampsb")
            nc.gpsimd.dma_start(out=samp_sb[:], in_=sampt[:])

            # ---- differentiation branch (all early; overlaps stream) ----
            psG = misc.tile([SN, SN], F32, tag="m")
            for k in range(NDC):
                nc.tensor.matmul(psG[:], samp_sb[:, k * SN:(k + 1) * SN],
                                 samp_sb[:, k * SN:(k + 1) * SN],
                                 start=(k == 0), stop=(k == NDC - 1))
            sqs = sb.tile([128, NDC * SN], F32, tag="sqs")
            nc.vector.tensor_tensor(sqs[:], samp_sb[:], samp_sb[:], OP.mult)
            psr = misc.tile([SN, 1], F32, tag="m")
            for k in range(NDC):
                nc.tensor.matmul(psr[:], sqs[:, k * SN:(k + 1) * SN],
                                 ones128[:], start=(k == 0),
                                 stop=(k == NDC - 1))
            g_sb = sb.tile([SN, SN], F32, tag="gsb")
            nc.scalar.copy(g_sb[:], psG[:])
            r_sb = sb.tile([SN, 1], F32, tag="rsb")
            nc.scalar.copy(r_sb[:], psr[:])

            # variance branch (DVE; early)
            mem3 = mem_sb[:].rearrange("p (k f) -> p k f", f=MEM)
            mean16 = sb.tile([128, NDC], F32, tag="mean16")
            nc.vector.tensor_reduce(mean16[:], mem3, AX.X, OP.add)
            nc.vector.tensor_scalar(mean16[:], mean16[:], 1.0 / MEM, None,
                                    OP.mult)
            cent = sb.tile([128, NDC * MEM], F32, tag="cent")
            nc.vector.tensor_tensor(
                cent[:].rearrange("p (k f) -> p k f", f=MEM), mem3,
                mean16[:, :, None].broadcast_to([128, NDC, MEM]), OP.subtract)
            nc.vector.tensor_tensor(cent[:], cent[:], cent[:], OP.mult)
            var16 = sb.tile([128, NDC], F32, tag="var16")
            nc.vector.tensor_reduce(
                var16[:], cent[:].rearrange("p (k f) -> p k f", f=MEM),
                AX.X, OP.add)
            nc.vector.tensor_scalar(var16[:], var16[:], 1.0 / (MEM - 1), None,
                                    OP.mult)
            redv = sb.tile([128, 1], F32, tag="redv")
            nc.vector.tensor_reduce(redv[:], var16[:], AX.X, OP.add)
            v2 = sb.tile([128, NDC], F32, tag="v2")
            nc.vector.tensor_tensor(v2[:], var16[:], var16[:], OP.mult)
            redv2 = sb.tile([128, 1], F32, tag="redv2")
            nc.vector.tensor_reduce(redv2[:], v2[:], AX.X, OP.add)
            pstv = misc.tile([1, 1], F32, tag="m")
            nc.tensor.matmul(pstv[:], redv[:], ones128[:], start=True,
                             stop=True)
            tv_sb = sb.tile([1, 1], F32, tag="tvsb")
            nc.scalar.copy(tv_sb[:], pstv[:])
            pss2 = misc.tile([1, 1], F32, tag="m")
            nc.tensor.matmul(pss2[:], redv2[:], ones128[:], start=True,
                             stop=True)
            s2_sb = sb.tile([1, 1], F32, tag="s2sb")
            nc.scalar.copy(s2_sb[:], pss2[:])

            tvsq = sb.tile([1, 1], F32, tag="tvsq")
            nc.vector.tensor_tensor(tvsq[:], tv_sb[:], tv_sb[:], OP.mult)
            dden = sb.tile([1, 1], F32, tag="dden")
            nc.vector.scalar_tensor_tensor(dden[:], tvsq[:], 1e-6, s2_sb[:],
                                           OP.mult, OP.add)
            rdden = sb.tile([1, 1], F32, tag="rdden")
            nc.vector.reciprocal(rdden[:], dden[:])
            eff_sb = sb.tile([1, 1], F32, tag="effsb")
            nc.vector.tensor_tensor(eff_sb[:], tvsq[:], rdden[:], OP.mult)

            # cdist tail: d2 = r_i + r_j - 2G
            rrow_ps = misc.tile([1, SN], F32, tag="m")
            nc.tensor.transpose(rrow_ps[:], r_sb[:], ident10[:])
            rrow = sb.tile([1, SN], F32, tag="rrow")
            nc.scalar.copy(rrow[:], rrow_ps[:])
            rB = misc.tile([SN, SN], F32, tag="m")
            nc.tensor.matmul(rB[:], ones1_10[:], rrow[:], start=True,
                             stop=True)
            d2 = sb.tile([SN, SN], F32, tag="d2")
            nc.vector.scalar_tensor_tensor(d2[:], g_sb[:], -2.0, rB[:],
                                           OP.mult, OP.add)
            nc.vector.tensor_scalar(d2[:], d2[:], r_sb[:], 0.0, OP.add,
                                    OP.max)
            dst = sb.tile([SN, SN], F32, tag="dst")
            nc.scalar.activation(dst[:], d2[:], ACT.Sqrt)
            dsum = sb.tile([SN, 1], F32, tag="dsum")
            nc.vector.tensor_reduce(dsum[:], dst[:], AX.X, OP.add)
            psD = misc.tile([1, 1], F32, tag="m")
            nc.tensor.matmul(psD[:], dsum[:], ones10[:], start=True, stop=True)
            avg_sb = sb.tile([1, 1], F32, tag="avgsb")
            nc.vector.tensor_scalar(avg_sb[:], psD[:],
                                    float(1.0 / (SN * (SN - 1) + 1e-6)), None,
                                    OP.mult)
            sqtv = sb.tile([1, 1], F32, tag="sqtv")
            nc.scalar.activation(sqtv[:], tv_sb[:], ACT.Sqrt)
            diff_sb = sb.tile([1, 1], F32, tag="diffsb")
            nc.vector.tensor_tensor(diff_sb[:], sqtv[:], avg_sb[:], OP.mult)
            tanhd = sb.tile([1, 1], F32, tag="tanhd")
            nc.scalar.activation(tanhd[:], diff_sb[:], ACT.Tanh)
            # load the Ln act table right after the last Tanh (input dep on
            # tanhd pins the scheduler) so no table switch hits the tail
            lnwarm = sb.tile([1, 1], F32, tag="lnwarm")
            nc.scalar.activation(lnwarm[:], tanhd[:], ACT.Ln)
            outrow = sb.tile([1, 9], F32, tag="outrow")
            nc.vector.tensor_copy(outrow[:, 1:2], diff_sb[:])
            nc.vector.tensor_copy(outrow[:, 2:3], eff_sb[:])
            nc.vector.tensor_copy(outrow[:, 3:4], tv_sb[:])

            # ---- stage A: stream HT; ht chunks stationary, masks moving ----
            # psAll[:, c*J+j] accumulates S.T[t, j] for t-chunk c: 128 t rows
            # on partitions, all 32 chunks x 8 series in half a PSUM bank.
            psAll = psA_pool.tile([128, NCH * J], F32, tag="sacc")
            for dk in range(NDC):
                if dk == 0:
                    htt = htt0
                elif dk == NDC - 1:
                    # halves on both queues so the tail after the last byte
                    # only covers 16 matmuls
                    htt = htp.tile([128, TL], F32, tag="htt", name="htt")
                    half = TL // 2
                    nc.sync.dma_start(out=htt[:, 0:half],
                                      in_=ht[dk * 128:(dk + 1) * 128, 0:half])
                    nc.gpsimd.dma_start(
                        out=htt[:, half:TL],
                        in_=ht[dk * 128:(dk + 1) * 128, half:TL])
                else:
                    htt = htp.tile([128, TL], F32, tag="htt", name="htt")
                    q = nc.sync if (dk % 2 == 0) else nc.gpsimd
                    q.dma_start(out=htt[:],
                                in_=ht[dk * 128:(dk + 1) * 128, :])
                for c in range(NCH):
                    # start zeroes the whole 2KB zero-region (bank), so only
                    # the very first matmul in the bank may carry start=True
                    nc.tensor.matmul(psAll[:, c * J:(c + 1) * J],
                                     htt[:, c * 128:(c + 1) * 128],
                                     m_sb[:, dk * J:(dk + 1) * J],
                                     start=(dk == 0 and c == 0),
                                     stop=(dk == NDC - 1 and c == NCH - 1),
                                     skip_group_check=True)

            # ---- stage B: raw min/max per series, scale, AllReduce(max) ----
            # mxmn cols 0:8 = max, cols 8:16 = -min (so one max-reduce after
            # transpose covers both); AR payload col1 carries invc (constant
            # across cores, so max is the identity on it)
            ps3 = psAll[:].rearrange("p (c j) -> p j c", j=J)
            mxmn = sb.tile([128, 2 * J], F32, tag="mxmn")
            nc.vector.tensor_reduce(mxmn[:, 0:J], ps3, AX.X, OP.max)
            nc.vector.tensor_reduce(mxmn[:, J:2 * J], ps3, AX.X, OP.min)
            nc.vector.tensor_scalar(mxmn[:, J:2 * J], mxmn[:, J:2 * J], -1.0,
                                    None, OP.mult)
            psT = misc.tile([2 * J, 128], F32, tag="m", name="psT")
            nc.tensor.transpose(psT[:], mxmn[:], ident128[:])
            minmax = sb.tile([2 * J, 2], F32, tag="minmax")
            nc.vector.tensor_copy(minmax[:, 1:2], invc_sb[:, 1:2])
            tmx = sb.tile([2 * J, 1], F32, tag="tmx")
            nc.vector.tensor_reduce(tmx[:], psT[:], AX.X, OP.max)
            nc.vector.tensor_scalar(minmax[:, 0:1], tmx[:],
                                    invc_sb[:, 0:1], None, OP.mult)
            cbA = dram.tile([2 * J, 2], F32, tag="cba")
            cbB = dram.tile([2 * J, 2], F32, tag="cbb")
            nc.sync.dma_start(out=cbA[:], in_=minmax[:])
            if sim1:
                nc.sync.dma_start(out=cbB[:], in_=cbA[:])
            else:
                nc.gpsimd.collective_compute("AllReduce", OP.max,
                                             replica_groups=rg,
                                             ins=[cbA.opt()],
                                             outs=[cbB.opt()])
            # read back replicated on every partition: grow[p, 2r+c]=cbB[r,c]
            grow = sb.tile([128, 4 * J], F32, tag="grow")
            nc.sync.dma_start(
                out=grow[:],
                in_=cbB[:].rearrange("r c -> (r c)")[None, :]
                .broadcast_to([128, 4 * J]))
            # row-wise: rng = max+(-min); s1 = 10/(rng+1e-6);
            # s1eff = s1*invc; b1 = (-min)*s1 - 0.5
            gmax = grow[:, 0:2 * J:2]
            gnmn = grow[:, 2 * J:4 * J:2]
            ginv = grow[:, 1:2 * J:2]
            rrow = sb.tile([128, J], F32, tag="rrow2")
            nc.vector.scalar_tensor_tensor(rrow[:], gmax, 1e-6, gnmn,
                                           OP.add, OP.add)
            nc.vector.reciprocal(rrow[:], rrow[:])
            s1eff = sb.tile([128, J], F32, tag="s1eff")
            nc.vector.tensor_tensor(s1eff[:], rrow[:], ginv, OP.mult)
            b1row = sb.tile([128, J], F32, tag="b1row")
            nc.vector.tensor_tensor(b1row[:], gnmn, rrow[:], OP.mult)
            nc.vector.tensor_scalar(b1row[:], b1row[:], 10.0, -0.5,
                                    OP.mult, OP.add)

            # ---- stage C: affine + int-cast + clamp + one-hot + joints ----
            binf = sb.tile([128, NCH * J], F32, tag="binf")
            b3 = binf[:].rearrange("p (c j) -> p c j", j=J)
            nc.vector.tensor_tensor(
                b3, psAll[:].rearrange("p (c j) -> p c j", j=J),
                s1eff[:, None, :].broadcast_to([128, NCH, J]), OP.mult)
            binint = sb.tile([128, NCH * J], I32, tag="binint")
            nc.vector.tensor_tensor(
                binint[:].rearrange("p (c j) -> p c j", j=J), b3,
                b1row[:, None, :].broadcast_to([128, NCH, J]),
                OP.add)
            nc.vector.tensor_scalar(binint[:], binint[:], 0, NB - 1, OP.max,
                                    OP.min)
            ohsb = sb.tile([128, NCH * J * NB], BF16, tag="ohsb")
            oh3 = ohsb[:].rearrange("p (c b) -> p c b", b=NB)
            for b in range(NB):
                eng = nc.vector if b < 7 else nc.gpsimd
                eng.tensor_scalar(oh3[:, :, b], binint[:], b, None,
                                  OP.is_equal)
            # joint histograms: all 4 pairs side by side in one PSUM bank
            psJt = psJ_pool.tile([NB, NPAIR * NB], F32, tag="pj")
            for c in range(NCH):
                for p in range(NPAIR):
                    xa = (c * J + 2 * p) * NB
                    ya = (c * J + 2 * p + 1) * NB
                    nc.tensor.matmul(psJt[:, p * NB:(p + 1) * NB],
                                     ohsb[:, xa:xa + NB],
                                     ohsb[:, ya:ya + NB],
                                     start=(c == 0 and p == 0),
                                     stop=(c == NCH - 1 and p == NPAIR - 1),
                                     skip_group_check=True)
            gjl = sb.tile([NB, NPAIR * NB], F32, tag="gjl")
            nc.vector.tensor_copy(gjl[:], psJt[:])
            cbj = dram.tile([NB, NPAIR * NB], F32, tag="cbj")
            cbj2 = dram.tile([NB, NPAIR * NB], F32, tag="cbj2")
            nc.sync.dma_start(out=cbj[:], in_=gjl[:])
            if sim1:
                nc.sync.dma_start(out=cbj2[:], in_=cbj[:])
            else:
                nc.gpsimd.collective_compute("AllReduce", OP.add,
                                             replica_groups=rg,
                                             ins=[cbj.opt()],
                                             outs=[cbj2.opt()])
            gj = sb.tile([NB, NPAIR * NB], F32, tag="gj")
            nc.sync.dma_start(out=gj[:], in_=cbj2[:])

            # ---- stage D: batched MI over the 4 pairs ----
            # mi_p = (1/T) sum_ij n_ij*(ln(n_ij+EPS_N)+LN_T-ln(r_i*c_j+EPS_RC))
            gj3 = gj[:].rearrange("a (p b) -> a p b", b=NB)
            r4 = sb.tile([NB, NPAIR], F32, tag="r4")
            nc.vector.tensor_reduce(r4[:], gj3, AX.X, OP.add)
            pscB = misc.tile([NB, NPAIR * NB], F32, tag="m", name="pscB")
            nc.tensor.matmul(pscB[:], ones10x10[:], gj[:], start=True,
                             stop=True)
            rc = sb.tile([NB, NPAIR * NB], F32, tag="rc")
            nc.vector.tensor_tensor(
                rc[:].rearrange("a (p b) -> a p b", b=NB),
                pscB[:].rearrange("a (p b) -> a p b", b=NB),
                r4[:, :, None].broadcast_to([NB, NPAIR, NB]), OP.mult)
            lnn = sb.tile([NB, NPAIR * NB], F32, tag="lnn")
            nc.scalar.activation(lnn[:], gj[:], ACT.Ln, bias=cepsn[:])
            lnrc = sb.tile([NB, NPAIR * NB], F32, tag="lnrc")
            nc.scalar.activation(lnrc[:], rc[:], ACT.Ln, bias=cepsrc[:])
            lterm = sb.tile([NB, NPAIR * NB], F32, tag="lterm")
            nc.vector.scalar_tensor_tensor(lterm[:], lnn[:], LN_T, lnrc[:],
                                           OP.add, OP.subtract)
            nc.vector.tensor_tensor(lterm[:], gj[:], lterm[:], OP.mult)
            rsum = sb.tile([NB, NPAIR], F32, tag="rsum")
            nc.vector.tensor_reduce(
                rsum[:], lterm[:].rearrange("a (p b) -> a p b", b=NB),
                AX.X, OP.add)
            psmi = misc.tile([1, NPAIR], F32, tag="m", name="psmi")
            nc.tensor.matmul(psmi[:], ones10[:], rsum[:], start=True,
                             stop=True)
            nc.vector.tensor_scalar(outrow[:, 5:9], psmi[:], INV_T, 0.0,
                                    OP.mult, OP.max)
            nc.vector.tensor_reduce(outrow[:, 4:5], outrow[:, 5:9], AX.X,
                                    OP.min)
            nc.vector.tensor_tensor(outrow[:, 0:1], outrow[:, 4:5], tanhd[:],
                                    OP.add)
            nc.sync.dma_start(out=out[:], in_=outrow[:])
            if debug:
                nc.sync.dma_start(out=dbg_st[:], in_=binf[:, 0:J])
                nc.sync.dma_start(out=dbg_gmm[:], in_=minmax[:])
                nc.sync.dma_start(out=dbg_s1b1[:, 0:J], in_=s1eff[0:1, :])
                nc.sync.dma_start(out=dbg_bin[:], in_=binint[:, 0:16])
                nc.sync.dma_start(out=dbg_gj[:], in_=gj[:])

    nc.compile()
    return nc


def _build_variant(name):
    return _build(variant=name)


def _get_nc(debug=False):
    key = ("ncd" if debug else "nc")
    if key not in _CACHE:
        _CACHE[key] = _build(debug)
    return _CACHE[key]


def kernel(state, state_memory, state_history, partitions, sample_idx,
           trace=False, debug=False):
    global LAST_RESULTS
    state = np.asarray(state, np.float32)
    state_memory = np.asarray(state_memory, np.float32)
    state_history = np.asarray(state_history, np.float32)
    partitions = np.asarray(partitions)
    sample_idx = np.asarray(sample_idx)

    mmat = np.empty((D, J), np.float32)
    invc8 = np.empty((J,), np.float32)
    pf = partitions.astype(np.float32)
    for p in range(NPAIR):
        mmat[:, 2 * p] = pf[p]
        mmat[:, 2 * p + 1] = np.float32(1.0) - pf[p]
        invc8[2 * p] = np.float32(1.0) / pf[p].sum(dtype=np.float32)
        invc8[2 * p + 1] = np.float32(1.0) / (np.float32(1.0) - pf[p]).sum(
            dtype=np.float32)
    invc = np.zeros((2 * J, 2), np.float32)
    invc[:, 0] = np.tile(invc8, 2)
    invc[0:J, 1] = np.float32(10.0) * invc8
    memory = np.concatenate([state, state_memory[state.shape[0]:]], axis=0)

    def _relayout(arrT, f):
        # [D, f] row-major -> [128, NDC*f]: row p holds chunks k at cols k*f
        return np.ascontiguousarray(
            arrT.reshape(NDC, 128, f).transpose(1, 0, 2).reshape(128, NDC * f))

    mmat = _relayout(mmat, J)
    memt = _relayout(np.ascontiguousarray(memory.T), MEM)
    sampt = _relayout(np.ascontiguousarray(memory[sample_idx].T), SN)

    in_maps = []
    for c in range(N_CORES):
        htc = np.ascontiguousarray(state_history[c * TL:(c + 1) * TL, :].T)
        in_maps.append({"ht": htc, "mmat": mmat, "invc": invc,
                        "memt": memt, "sampt": sampt})

    nc = _get_nc(debug)
    res = run_bass_kernel_spmd(nc, in_maps, list(range(N_CORES)),
                               trace=trace)
    LAST_RESULTS = res
    return np.asarray(res.results[0]["out"], np.float32)


# revision 21
# speedup vs baseline: 1.4836x; 1.0077x over previous
"""Trainium2 Bass kernel for nn_ConsciousnessMonitor (histogram_binning).

kernel(**inputs) takes FULL unsharded numpy inputs, returns the full (9,)
float32 output. Shards state_history along time across 8 NeuronCores:
masked means via PE matmul while streaming (ht chunks stationary, mask
columns moving, so S arrives time-major), min/max + joint-histogram MI
with two small AllReduces, differentiation branch replicated per core.

Self-contained: shapes/sharding hardcoded; reads no sibling files.
"""
import numpy as np
import ml_dtypes

import concourse.bacc as bacc
import concourse.tile as tile
import concourse.mybir as mybir
from concourse.bass_utils import run_bass_kernel_spmd
from concourse.masks import make_identity

F32 = mybir.dt.float32
I32 = mybir.dt.int32
BF16 = mybir.dt.bfloat16
AX = mybir.AxisListType
OP = mybir.AluOpType
ACT = mybir.ActivationFunctionType

N_CORES = 8
T, D = 32768, 2048
TL = T // N_CORES          # 4096 time steps per core
NB = 10                    # histogram bins per axis
NPAIR = 4                  # partitions (mask pairs)
J = 2 * NPAIR              # 8 masked-mean columns
NDC = D // 128             # 16 contraction chunks
NCH = TL // 128            # 32 time chunks of 128 (PSUM cols / binning)
MEM = 100
SN = 10

LN_T = float(np.log(np.float32(T)))
INV_T = 1.0 / T
EPS_N = T * 1e-10          # joint-count epsilon under common denominator
EPS_RC = float(T) * T * 1e-10  # outer-product epsilon likewise

_CACHE = {}
LAST_RESULTS = None


def _build(debug=False, variant="main"):
    sim1 = variant.startswith("sim1")
    nc = bacc.Bacc("TRN2", target_bir_lowering=False, debug=False,
                   num_devices=1 if sim1 else N_CORES)
    ht = nc.dram_tensor("ht", [D, TL], F32, kind="ExternalInput").ap()
    mmat = nc.dram_tensor("mmat", [128, NDC * J], F32,
                          kind="ExternalInput").ap()
    invc = nc.dram_tensor("invc", [2 * J, 2], F32,
                      kind="ExternalInput").ap()
    memt = nc.dram_tensor("memt", [128, NDC * MEM], BF16,
                          kind="ExternalInput").ap()
    sampt = nc.dram_tensor("sampt", [128, NDC * SN], BF16,
                           kind="ExternalInput").ap()
    out = nc.dram_tensor("out", [9], F32, kind="ExternalOutput").ap()
    if debug:
        dbg_st = nc.dram_tensor("dbg_st", [128, J], F32,
                                kind="ExternalOutput").ap()
        dbg_gmm = nc.dram_tensor("dbg_gmm", [2 * J, 2], F32,
                                 kind="ExternalOutput").ap()
        dbg_s1b1 = nc.dram_tensor("dbg_s1b1", [1, 2 * J], F32,
                                  kind="ExternalOutput").ap()
        dbg_bin = nc.dram_tensor("dbg_bin", [128, 16], I32,
                                 kind="ExternalOutput").ap()
        dbg_gj = nc.dram_tensor("dbg_gj", [NB, NPAIR * NB], F32,
                                kind="ExternalOutput").ap()

    rg = [list(range(N_CORES))]

    with tile.TileContext(nc) as tc:
        with tc.tile_pool(name="consts", bufs=1) as consts, \
             tc.tile_pool(name="sb", bufs=1) as sb, \
             tc.tile_pool(name="htp", bufs=2) as htp, \
             tc.tile_pool(name="psA", bufs=1, space="PSUM") as psA_pool, \
             tc.tile_pool(name="psJ", bufs=1, space="PSUM") as psJ_pool, \
             tc.tile_pool(name="misc", bufs=3, space="PSUM") as misc, \
             tc.tile_pool(name="dram", bufs=1, space="DRAM") as dram:

            # ---- constants / small inputs ----
            ident10 = consts.tile([NB, NB], F32, tag="id10")
            make_identity(nc, ident10[:])
            ident128 = consts.tile([128, 128], F32, tag="id128")
            make_identity(nc, ident128[:])
            ones128 = consts.tile([128, 1], F32, tag="o128")
            nc.gpsimd.memset(ones128[:], 1.0)
            ones10 = consts.tile([NB, 1], F32, tag="o10")
            nc.gpsimd.memset(ones10[:], 1.0)
            ones1_10 = consts.tile([1, NB], F32, tag="o110")
            nc.gpsimd.memset(ones1_10[:], 1.0)
            ones1_128 = consts.tile([1, 128], F32, tag="o1128")
            nc.gpsimd.memset(ones1_128[:], 1.0)
            ones10x10 = consts.tile([NB, NB], F32, tag="o1010")
            nc.gpsimd.memset(ones10x10[:], 1.0)

            cepsrc = consts.tile([NB, 1], F32, tag="cepsrc")
            nc.gpsimd.memset(cepsrc[:], EPS_RC)
            cepsn = consts.tile([NB, 1], F32, tag="cepsn")
            nc.gpsimd.memset(cepsn[:], EPS_N)

            htt0 = htp.tile([128, TL], F32, tag="htt", name="htt")
            nc.sync.dma_start(out=htt0[:], in_=ht[0:128, :])
            m_sb = consts.tile([128, NDC * J], F32, tag="msb")
            nc.sync.dma_start(out=m_sb[:], in_=mmat[:])
            invc_sb = consts.tile([2 * J, 2], F32, tag="invc")
            nc.gpsimd.dma_start(out=invc_sb[:], in_=invc[:])
            mem_sb = consts.tile([128, NDC * MEM], BF16, tag="memsb")
            nc.gpsimd.dma_start(out=mem_sb[:], in_=memt[:])
            samp_sb = consts.tile([128, NDC * SN], BF16, tag="sampsb")
            nc.gpsimd.dma_start(out=samp_sb[:], in_=sampt[:])

            # ---- differentiation branch (all early; overlaps stream) ----
            psG = misc.tile([SN, SN], F32, tag="m")
            for k in range(NDC):
                nc.tensor.matmul(psG[:], samp_sb[:, k * SN:(k + 1) * SN],
                                 samp_sb[:, k * SN:(k + 1) * SN],
                                 start=(k == 0), stop=(k == NDC - 1))
            sqs = sb.tile([128, NDC * SN], F32, tag="sqs")
            nc.vector.tensor_tensor(sqs[:], samp_sb[:], samp_sb[:], OP.mult)
            psr = misc.tile([SN, 1], F32, tag="m")
            for k in range(NDC):
                nc.tensor.matmul(psr[:], sqs[:, k * SN:(k + 1) * SN],
                                 ones128[:], start=(k == 0),
                                 stop=(k == NDC - 1))
            g_sb = sb.tile([SN, SN], F32, tag="gsb")
            nc.scalar.copy(g_sb[:], psG[:])
            r_sb = sb.tile([SN, 1], F32, tag="rsb")
            nc.scalar.copy(r_sb[:], psr[:])

            # variance branch (DVE; early)
            mem3 = mem_sb[:].rearrange("p (k f) -> p k f", f=MEM)
            mean16 = sb.tile([128, NDC], F32, tag="mean16")
            nc.vector.tensor_reduce(mean16[:], mem3, AX.X, OP.add)
            nc.vector.tensor_scalar(mean16[:], mean16[:], 1.0 / MEM, None,
                                    OP.mult)
            cent = sb.tile([128, NDC * MEM], F32, tag="cent")
            nc.vector.tensor_tensor(
                cent[:].rearrange("p (k f) -> p k f", f=MEM), mem3,
                mean16[:, :, None].broadcast_to([128, NDC, MEM]), OP.subtract)
            nc.vector.tensor_tensor(cent[:], cent[:], cent[:], OP.mult)
            var16 = sb.tile([128, NDC], F32, tag="var16")
            nc.vector.tensor_reduce(
                var16[:], cent[:].rearrange("p (k f) -> p k f", f=MEM),
                AX.X, OP.add)
            nc.vector.tensor_scalar(var16[:], var16[:], 1.0 / (MEM - 1), None,
                                    OP.mult)
            redv = sb.tile([128, 1], F32, tag="redv")
            nc.vector.tensor_reduce(redv[:], var16[:], AX.X, OP.add)
            v2 = sb.tile([128, NDC], F32, tag="v2")
            nc.vector.tensor_tensor(v2[:], var16[:], var16[:], OP.mult)
            redv2 = sb.tile([128, 1], F32, tag="redv2")
            nc.vector.tensor_reduce(redv2[:], v2[:], AX.X, OP.add)
            pstv = misc.tile([1, 1], F32, tag="m")
            nc.tensor.matmul(pstv[:], redv[:], ones128[:], start=True,
                             stop=True)
            tv_sb = sb.tile([1, 1], F32, tag="tvsb")
            nc.scalar.copy(tv_sb[:], pstv[:])
            pss2 = misc.tile([1, 1], F32, tag="m")
            nc.tensor.matmul(pss2[:], redv2[:], ones128[:], start=True,
                             stop=True)
            s2_sb = sb.tile([1, 1], F32, tag="s2sb")
            nc.scalar.copy(s2_sb[:], pss2[:])

            tvsq = sb.tile([1, 1], F32, tag="tvsq")
            nc.vector.tensor_tensor(tvsq[:], tv_sb[:], tv_sb[:], OP.mult)
            dden = sb.tile([1, 1], F32, tag="dden")
            nc.vector.scalar_tensor_tensor(dden[:], tvsq[:], 1e-6, s2_sb[:],
                                           OP.mult, OP.add)
            rdden = sb.tile([1, 1], F32, tag="rdden")
            nc.vector.reciprocal(rdden[:], dden[:])
            eff_sb = sb.tile([1, 1], F32, tag="effsb")
            nc.vector.tensor_tensor(eff_sb[:], tvsq[:], rdden[:], OP.mult)

            # cdist tail: d2 = r_i + r_j - 2G
            rrow_ps = misc.tile([1, SN], F32, tag="m")
            nc.tensor.transpose(rrow_ps[:], r_sb[:], ident10[:])
            rrow = sb.tile([1, SN], F32, tag="rrow")
            nc.scalar.copy(rrow[:], rrow_ps[:])
            rB = misc.tile([SN, SN], F32, tag="m")
            nc.tensor.matmul(rB[:], ones1_10[:], rrow[:], start=True,
                             stop=True)
            d2 = sb.tile([SN, SN], F32, tag="d2")
            nc.vector.scalar_tensor_tensor(d2[:], g_sb[:], -2.0, rB[:],
                                           OP.mult, OP.add)
            nc.vector.tensor_scalar(d2[:], d2[:], r_sb[:], 0.0, OP.add,
                                    OP.max)
            dst = sb.tile([SN, SN], F32, tag="dst")
            nc.scalar.activation(dst[:], d2[:], ACT.Sqrt)
            dsum = sb.tile([SN, 1], F32, tag="dsum")
            nc.vector.tensor_reduce(dsum[:], dst[:], AX.X, OP.add)
            psD = misc.tile([1, 1], F32, tag="m")
            nc.tensor.matmul(psD[:], dsum[:], ones10[:], start=True, stop=True)
            avg_sb = sb.tile([1, 1], F32, tag="avgsb")
            nc.vector.tensor_scalar(avg_sb[:], psD[:],
                                    float(1.0 / (SN * (SN - 1) + 1e-6)), None,
                                    OP.mult)
            sqtv = sb.tile([1, 1], F32, tag="sqtv")
            nc.scalar.activation(sqtv[:], tv_sb[:], ACT.Sqrt)
            diff_sb = sb.tile([1, 1], F32, tag="diffsb")
            nc.vector.tensor_tensor(diff_sb[:], sqtv[:], avg_sb[:], OP.mult)
            tanhd = sb.tile([1, 1], F32, tag="tanhd")
            nc.scalar.activation(tanhd[:], diff_sb[:], ACT.Tanh)
            # load the Ln act table right after the last Tanh (input dep on
            # tanhd pins the scheduler) so no table switch hits the tail
            lnwarm = sb.tile([1, 1], F32, tag="lnwarm")
            nc.scalar.activation(lnwarm[:], tanhd[:], ACT.Ln)
            outrow = sb.tile([1, 9], F32, tag="outrow")
            nc.vector.tensor_copy(outrow[:, 1:2], diff_sb[:])
            nc.vector.tensor_copy(outrow[:, 2:3], eff_sb[:])
            nc.vector.tensor_copy(outrow[:, 3:4], tv_sb[:])

            # ---- stage A: stream HT; ht chunks stationary, masks moving ----
            # psAll[:, c*J+j] accumulates S.T[t, j] for t-chunk c: 128 t rows
            # on partitions, all 32 chunks x 8 series in half a PSUM bank.
            psAll = psA_pool.tile([128, NCH * J], F32, tag="sacc")
            for dk in range(NDC):
                if dk == 0:
                    htt = htt0
                elif dk == NDC - 1:
                    # halves on both queues so the tail after the last byte
                    # only covers 16 matmuls
                    htt = htp.tile([128, TL], F32, tag="htt", name="htt")
                    half = TL // 2
                    nc.sync.dma_start(out=htt[:, 0:half],
                                      in_=ht[dk * 128:(dk + 1) * 128, 0:half])
                    nc.gpsimd.dma_start(
                        out=htt[:, half:TL],
                        in_=ht[dk * 128:(dk + 1) * 128, half:TL])
                else:
                    htt = htp.tile([128, TL], F32, tag="htt", name="htt")
                    q = nc.sync if (dk % 2 == 0) else nc.gpsimd
                    q.dma_start(out=htt[:],
                                in_=ht[dk * 128:(dk + 1) * 128, :])
                for c in range(NCH):
                    # start zeroes the whole 2KB zero-region (bank), so only
                    # the very first matmul in the bank may carry start=True
                    nc.tensor.matmul(psAll[:, c * J:(c + 1) * J],
                                     htt[:, c * 128:(c + 1) * 128],
                                     m_sb[:, dk * J:(dk + 1) * J],
                                     start=(dk == 0 and c == 0),
                                     stop=(dk == NDC - 1 and c == NCH - 1),
                                     skip_group_check=True)

            # ---- stage B: raw min/max per series, scale, AllReduce(max) ----
            # mxmn cols 0:8 = max, cols 8:16 = -min (so one max-reduce after
            # transpose covers both); AR payload col1 carries invc (constant
            # across cores, so max is the identity on it)
            ps3 = psAll[:].rearrange("p (c j) -> p j c", j=J)
            mxmn = sb.tile([128, 2 * J], F32, tag="mxmn")
            nc.vector.tensor_reduce(mxmn[:, 0:J], ps3, AX.X, OP.max)
            nc.vector.tensor_reduce(mxmn[:, J:2 * J], ps3, AX.X, OP.min)
            nc.vector.tensor_scalar(mxmn[:, J:2 * J], mxmn[:, J:2 * J], -1.0,
                                    None, OP.mult)
            psT = misc.tile([2 * J, 128], F32, tag="m", name="psT")
            nc.tensor.transpose(psT[:], mxmn[:], ident128[:])
            minmax = sb.tile([2 * J, 2], F32, tag="minmax")
            nc.vector.tensor_copy(minmax[:, 1:2], invc_sb[:, 1:2])
            tmx = sb.tile([2 * J, 1], F32, tag="tmx")
            nc.vector.tensor_reduce(tmx[:], psT[:], AX.X, OP.max)
            nc.vector.tensor_scalar(minmax[:, 0:1], tmx[:],
                                    invc_sb[:, 0:1], None, OP.mult)
            cbA = dram.tile([2 * J, 2], F32, tag="cba")
            cbB = dram.tile([2 * J, 2], F32, tag="cbb")
            nc.sync.dma_start(out=cbA[:], in_=minmax[:])
            if sim1:
                nc.sync.dma_start(out=cbB[:], in_=cbA[:])
            else:
                nc.gpsimd.collective_compute("AllReduce", OP.max,
                                             replica_groups=rg,
                                             ins=[cbA.opt()],
                                             outs=[cbB.opt()])
            # read back replicated on every partition: grow[p, 2r+c]=cbB[r,c]
            grow = sb.tile([128, 4 * J], F32, tag="grow")
            nc.sync.dma_start(
                out=grow[:],
                in_=cbB[:].rearrange("r c -> (r c)")[None, :]
                .broadcast_to([128, 4 * J]))
            # row-wise: rng = max+(-min); s1 = 10/(rng+1e-6);
            # s1eff = s1*invc; b1 = (-min)*s1 - 0.5
            gmax = grow[:, 0:2 * J:2]
            gnmn = grow[:, 2 * J:4 * J:2]
            ginv = grow[:, 1:2 * J:2]
            rrow = sb.tile([128, J], F32, tag="rrow2")
            nc.vector.scalar_tensor_tensor(rrow[:], gmax, 1e-6, gnmn,
                                           OP.add, OP.add)
            nc.vector.reciprocal(rrow[:], rrow[:])
            s1eff = sb.tile([128, J], F32, tag="s1eff")
            nc.vector.tensor_tensor(s1eff[:], rrow[:], ginv, OP.mult)
            b1row = sb.tile([128, J], F32, tag="b1row")
            nc.vector.tensor_tensor(b1row[:], gnmn, rrow[:], OP.mult)
            nc.vector.tensor_scalar(b1row[:], b1row[:], 10.0, -0.5,
                                    OP.mult, OP.add)

            # ---- stage C: affine + int-cast + clamp + one-hot + joints ----
            binf = sb.tile([128, NCH * J], F32, tag="binf")
            b3 = binf[:].rearrange("p (c j) -> p c j", j=J)
            nc.vector.tensor_tensor(
                b3, psAll[:].rearrange("p (c j) -> p c j", j=J),
                s1eff[:, None, :].broadcast_to([128, NCH, J]), OP.mult)
            binint = sb.tile([128, NCH * J], I32, tag="binint")
            nc.vector.tensor_tensor(
                binint[:].rearrange("p (c j) -> p c j", j=J), b3,
                b1row[:, None, :].broadcast_to([128, NCH, J]),
                OP.add)
            nc.vector.tensor_scalar(binint[:], binint[:], 0, NB - 1, OP.max,
                                    OP.min)
            ohsb = sb.tile([128, NCH * J * NB], BF16, tag="ohsb")
            oh3 = ohsb[:].rearrange("p (c b) -> p c b", b=NB)
            for b in range(NB):
                eng = nc.vector if b < 7 else nc.gpsimd
                eng.tensor_scalar(oh3[:, :, b], binint[:], b, None,
                                  OP.is_equal)
            # joint histograms: all 4 pairs side by side in one PSUM bank
            psJt = psJ_pool.tile([NB, NPAIR * NB], F32, tag="pj")
            for c in range(NCH):
                for p in range(NPAIR):
                    xa = (c * J + 2 * p) * NB
                    ya = (c * J + 2 * p + 1) * NB
                    nc.tensor.matmul(psJt[:, p * NB:(p + 1) * NB],
                                     ohsb[:, xa:xa + NB],
                                     ohsb[:, ya:ya + NB],
                                     start=(c == 0 and p == 0),
                                     stop=(c == NCH - 1 and p == NPAIR - 1),
                                     skip_group_check=True)
            gjl = sb.tile([NB, NPAIR * NB], F32, tag="gjl")
            nc.vector.tensor_copy(gjl[:], psJt[:])
            cbj = dram.tile([NB, NPAIR * NB], F32, tag="cbj")
            cbj2 = dram.tile([NB, NPAIR * NB], F32, tag="cbj2")
            nc.sync.dma_start(out=cbj[:], in_=gjl[:])
            if sim1:
                nc.sync.dma_start(out=cbj2[:], in_=cbj[:])
            else:
                nc.gpsimd.collective_compute("AllReduce", OP.add,
                                             replica_groups=rg,
                                             ins=[cbj.opt()],
                                             outs=[cbj2.opt()])
            gj = sb.tile([NB, NPAIR * NB], F32, tag="gj")
            nc.sync.dma_start(out=gj[:], in_=cbj2[:])

            # ---- stage D: batched MI over the 4 pairs ----
            # mi_p = (1/T) sum_ij n_ij*(ln(n_ij+EPS_N)+LN_T-ln(r_i*c_j+EPS_RC))
            gj3 = gj[:].rearrange("a (p b) -> a p b", b=NB)
            r4 = sb.tile([NB, NPAIR], F32, tag="r4")
            nc.vector.tensor_reduce(r4[:], gj3, AX.X, OP.add)
            pscB = misc.tile([NB, NPAIR * NB], F32, tag="m", name="pscB")
            nc.tensor.matmul(pscB[:], ones10x10[:], gj[:], start=True,
                             stop=True)
            rc = sb.tile([NB, NPAIR * NB], F32, tag="rc")
            nc.vector.tensor_tensor(
                rc[:].rearrange("a (p b) -> a p b", b=NB),
                pscB[:].rearrange("a (p b) -> a p b", b=NB),
                r4[:, :, None].broadcast_to([NB, NPAIR, NB]), OP.mult)
            lnn = sb.tile([NB, NPAIR * NB], F32, tag="lnn")
            nc.scalar.activation(lnn[:], gj[:], ACT.Ln, bias=cepsn[:])
            lnrc = sb.tile([NB, NPAIR * NB], F32, tag="lnrc")
            nc.scalar.activation(lnrc[:], rc[:], ACT.Ln, bias=cepsrc[:])
            lterm = sb.tile([NB, NPAIR * NB], F32, tag="lterm")
            nc.vector.scalar_tensor_tensor(lterm[:], lnn[:], LN_T, lnrc[:],
                                           OP.add, OP.subtract)
            nc.vector.tensor_tensor(lterm[:], gj[:], lterm[:], OP.mult)
            rsum = sb.tile([NB, NPAIR], F32, tag="rsum")
            nc.vector.tensor_reduce(
                rsum[:], lterm[:].rearrange("a (p b) -> a p b", b=NB),
                AX.X, OP.add)
            psmi = misc.tile([1, NPAIR], F32, tag="m", name="psmi")
            nc.tensor.matmul(psmi[:], ones10[:], rsum[:], start=True,
                             stop=True)
            nc.vector.tensor_scalar(outrow[:, 5:9], psmi[:], INV_T, 0.0,
                                    OP.mult, OP.max)
            nc.vector.tensor_reduce(outrow[:, 4:5], outrow[:, 5:9], AX.X,
                                    OP.min)
            nc.vector.tensor_tensor(outrow[:, 0:1], outrow[:, 4:5], tanhd[:],
                                    OP.add)
            nc.sync.dma_start(out=out[:], in_=outrow[:])
            if debug:
                nc.sync.dma_start(out=dbg_st[:], in_=binf[:, 0:J])
                nc.sync.dma_start(out=dbg_gmm[:], in_=minmax[:])
                nc.sync.dma_start(out=dbg_s1b1[:, 0:J], in_=s1eff[0:1, :])
                nc.sync.dma_start(out=dbg_bin[:], in_=binint[:, 0:16])
                nc.sync.dma_start(out=dbg_gj[:], in_=gj[:])

    nc.compile()
    return nc


def _build_variant(name):
    return _build(variant=name)


def _get_nc(debug=False):
    key = ("ncd" if debug else "nc")
    if key not in _CACHE:
        _CACHE[key] = _build(debug)
    return _CACHE[key]


def kernel(state, state_memory, state_history, partitions, sample_idx,
           trace=False, debug=False):
    global LAST_RESULTS
    state = np.asarray(state, np.float32)
    state_memory = np.asarray(state_memory, np.float32)
    state_history = np.asarray(state_history, np.float32)
    partitions = np.asarray(partitions)
    sample_idx = np.asarray(sample_idx)

    mmat = np.empty((D, J), np.float32)
    invc8 = np.empty((J,), np.float32)
    pf = partitions.astype(np.float32)
    for p in range(NPAIR):
        mmat[:, 2 * p] = pf[p]
        mmat[:, 2 * p + 1] = np.float32(1.0) - pf[p]
        invc8[2 * p] = np.float32(1.0) / pf[p].sum(dtype=np.float32)
        invc8[2 * p + 1] = np.float32(1.0) / (np.float32(1.0) - pf[p]).sum(
            dtype=np.float32)
    invc = np.zeros((2 * J, 2), np.float32)
    invc[:, 0] = np.tile(invc8, 2)
    invc[0:J, 1] = np.float32(10.0) * invc8
    memory = np.concatenate([state, state_memory[state.shape[0]:]], axis=0)

    def _relayout(arrT, f):
        # [D, f] row-major -> [128, NDC*f]: row p holds chunks k at cols k*f
        return np.ascontiguousarray(
            arrT.reshape(NDC, 128, f).transpose(1, 0, 2).reshape(128, NDC * f))

    mmat = _relayout(mmat, J)
    memt = _relayout(np.ascontiguousarray(memory.T), MEM).astype(
        ml_dtypes.bfloat16)
    sampt = _relayout(np.ascontiguousarray(memory[sample_idx].T), SN).astype(
        ml_dtypes.bfloat16)

    in_maps = []
    for c in range(N_CORES):
        htc = np.ascontiguousarray(state_history[c * TL:(c + 1) * TL, :].T)
        in_maps.append({"ht": htc, "mmat": mmat, "invc": invc,
                        "memt": memt, "sampt": sampt})

    nc = _get_nc(debug)
    res = run_bass_kernel_spmd(nc, in_maps, list(range(N_CORES)),
                               trace=trace)
    LAST_RESULTS = res
    return np.asarray(res.results[0]["out"], np.float32)


# revision 27
# speedup vs baseline: 1.4854x; 1.0012x over previous
"""Trainium2 Bass kernel for nn_ConsciousnessMonitor (histogram_binning).

kernel(**inputs) takes FULL unsharded numpy inputs, returns the full (9,)
float32 output. Shards state_history along time across 8 NeuronCores:
masked means via PE matmul while streaming (ht chunks stationary, mask
columns moving, so S arrives time-major), min/max + joint-histogram MI
with two small AllReduces, differentiation branch replicated per core.

Self-contained: shapes/sharding hardcoded; reads no sibling files.
"""
import numpy as np
import ml_dtypes

import concourse.bacc as bacc
import concourse.tile as tile
import concourse.mybir as mybir
from concourse.bass_utils import run_bass_kernel_spmd
from concourse.masks import make_identity

F32 = mybir.dt.float32
I32 = mybir.dt.int32
BF16 = mybir.dt.bfloat16
AX = mybir.AxisListType
OP = mybir.AluOpType
ACT = mybir.ActivationFunctionType

N_CORES = 8
T, D = 32768, 2048
TL = T // N_CORES          # 4096 time steps per core
NB = 10                    # histogram bins per axis
NPAIR = 4                  # partitions (mask pairs)
J = 2 * NPAIR              # 8 masked-mean columns
NDC = D // 128             # 16 contraction chunks
NCH = TL // 128            # 32 time chunks of 128 (PSUM cols / binning)
MEM = 100
SN = 10

LN_T = float(np.log(np.float32(T)))
INV_T = 1.0 / T
EPS_N = T * 1e-10          # joint-count epsilon under common denominator
EPS_RC = float(T) * T * 1e-10  # outer-product epsilon likewise

_CACHE = {}
LAST_RESULTS = None


def _build(debug=False, variant="main"):
    sim1 = variant.startswith("sim1")
    nc = bacc.Bacc("TRN2", target_bir_lowering=False, debug=False,
                   num_devices=1 if sim1 else N_CORES)
    ht = nc.dram_tensor("ht", [D, TL], F32, kind="ExternalInput").ap()
    mmat = nc.dram_tensor("mmat", [128, NDC * J], F32,
                          kind="ExternalInput").ap()
    invc = nc.dram_tensor("invc", [2 * J, 2], F32,
                      kind="ExternalInput").ap()
    memt = nc.dram_tensor("memt", [128, NDC * MEM], BF16,
                          kind="ExternalInput").ap()
    sampt = nc.dram_tensor("sampt", [128, NDC * SN], BF16,
                           kind="ExternalInput").ap()
    out = nc.dram_tensor("out", [9], F32, kind="ExternalOutput").ap()
    if debug:
        dbg_st = nc.dram_tensor("dbg_st", [128, J], F32,
                                kind="ExternalOutput").ap()
        dbg_gmm = nc.dram_tensor("dbg_gmm", [2 * J, 2], F32,
                                 kind="ExternalOutput").ap()
        dbg_s1b1 = nc.dram_tensor("dbg_s1b1", [1, 2 * J], F32,
                                  kind="ExternalOutput").ap()
        dbg_bin = nc.dram_tensor("dbg_bin", [128, 16], I32,
                                 kind="ExternalOutput").ap()
        dbg_gj = nc.dram_tensor("dbg_gj", [NB, NPAIR * NB], F32,
                                kind="ExternalOutput").ap()

    rg = [list(range(N_CORES))]

    with tile.TileContext(nc) as tc:
        with tc.tile_pool(name="consts", bufs=1) as consts, \
             tc.tile_pool(name="sb", bufs=1) as sb, \
             tc.tile_pool(name="htp", bufs=2) as htp, \
             tc.tile_pool(name="psA", bufs=1, space="PSUM") as psA_pool, \
             tc.tile_pool(name="psJ", bufs=1, space="PSUM") as psJ_pool, \
             tc.tile_pool(name="misc", bufs=3, space="PSUM") as misc, \
             tc.tile_pool(name="dram", bufs=1, space="DRAM") as dram:

            # ---- constants / small inputs ----
            ident10 = consts.tile([NB, NB], F32, tag="id10")
            make_identity(nc, ident10[:])
            ident128 = consts.tile([128, 128], F32, tag="id128")
            make_identity(nc, ident128[:])
            ones128 = consts.tile([128, 1], F32, tag="o128")
            nc.gpsimd.memset(ones128[:], 1.0)
            ones10 = consts.tile([NB, 1], F32, tag="o10")
            nc.gpsimd.memset(ones10[:], 1.0)
            ones1_10 = consts.tile([1, NB], F32, tag="o110")
            nc.gpsimd.memset(ones1_10[:], 1.0)
            ones1_128 = consts.tile([1, 128], F32, tag="o1128")
            nc.gpsimd.memset(ones1_128[:], 1.0)
            ones10x10 = consts.tile([NB, NB], F32, tag="o1010")
            nc.gpsimd.memset(ones10x10[:], 1.0)

            cepsrc = consts.tile([NB, 1], F32, tag="cepsrc")
            nc.gpsimd.memset(cepsrc[:], EPS_RC)
            cepsn = consts.tile([NB, 1], F32, tag="cepsn")
            nc.gpsimd.memset(cepsn[:], EPS_N)

            htt0 = htp.tile([128, TL], F32, tag="htt", name="htt")
            nc.sync.dma_start(out=htt0[:], in_=ht[0:128, :])
            m_sb = consts.tile([128, NDC * J], F32, tag="msb")
            nc.sync.dma_start(out=m_sb[:], in_=mmat[:])
            invc_sb = consts.tile([2 * J, 2], F32, tag="invc")
            nc.gpsimd.dma_start(out=invc_sb[:], in_=invc[:])
            mem_sb = consts.tile([128, NDC * MEM], BF16, tag="memsb")
            nc.gpsimd.dma_start(out=mem_sb[:], in_=memt[:])
            samp_sb = consts.tile([128, NDC * SN], BF16, tag="sampsb")
            nc.gpsimd.dma_start(out=samp_sb[:], in_=sampt[:])

            # ---- differentiation branch (all early; overlaps stream) ----
            psG = misc.tile([SN, SN], F32, tag="m")
            for k in range(NDC):
                nc.tensor.matmul(psG[:], samp_sb[:, k * SN:(k + 1) * SN],
                                 samp_sb[:, k * SN:(k + 1) * SN],
                                 start=(k == 0), stop=(k == NDC - 1))
            sqs = sb.tile([128, NDC * SN], F32, tag="sqs")
            nc.vector.tensor_tensor(sqs[:], samp_sb[:], samp_sb[:], OP.mult)
            psr = misc.tile([SN, 1], F32, tag="m")
            for k in range(NDC):
                nc.tensor.matmul(psr[:], sqs[:, k * SN:(k + 1) * SN],
                                 ones128[:], start=(k == 0),
                                 stop=(k == NDC - 1))
            g_sb = sb.tile([SN, SN], F32, tag="gsb")
            nc.scalar.copy(g_sb[:], psG[:])
            r_sb = sb.tile([SN, 1], F32, tag="rsb")
            nc.scalar.copy(r_sb[:], psr[:])

            # variance branch (DVE; early)
            mem3 = mem_sb[:].rearrange("p (k f) -> p k f", f=MEM)
            mean16 = sb.tile([128, NDC], F32, tag="mean16")
            nc.vector.tensor_reduce(mean16[:], mem3, AX.X, OP.add)
            nc.vector.tensor_scalar(mean16[:], mean16[:], 1.0 / MEM, None,
                                    OP.mult)
            cent = sb.tile([128, NDC * MEM], F32, tag="cent")
            nc.vector.tensor_tensor(
                cent[:].rearrange("p (k f) -> p k f", f=MEM), mem3,
                mean16[:, :, None].broadcast_to([128, NDC, MEM]), OP.subtract)
            nc.vector.tensor_tensor(cent[:], cent[:], cent[:], OP.mult)
            var16 = sb.tile([128, NDC], F32, tag="var16")
            nc.vector.tensor_reduce(
                var16[:], cent[:].rearrange("p (k f) -> p k f", f=MEM),
                AX.X, OP.add)
            nc.vector.tensor_scalar(var16[:], var16[:], 1.0 / (MEM - 1), None,
                                    OP.mult)
            redv = sb.tile([128, 1], F32, tag="redv")
            nc.vector.tensor_reduce(redv[:], var16[:], AX.X, OP.add)
            v2 = sb.tile([128, NDC], F32, tag="v2")
            nc.vector.tensor_tensor(v2[:], var16[:], var16[:], OP.mult)
            redv2 = sb.tile([128, 1], F32, tag="redv2")
            nc.vector.tensor_reduce(redv2[:], v2[:], AX.X, OP.add)
            pstv = misc.tile([1, 1], F32, tag="m")
            nc.tensor.matmul(pstv[:], redv[:], ones128[:], start=True,
                             stop=True)
            tv_sb = sb.tile([1, 1], F32, tag="tvsb")
            nc.scalar.copy(tv_sb[:], pstv[:])
            pss2 = misc.tile([1, 1], F32, tag="m")
            nc.tensor.matmul(pss2[:], redv2[:], ones128[:], start=True,
                             stop=True)
            s2_sb = sb.tile([1, 1], F32, tag="s2sb")
            nc.scalar.copy(s2_sb[:], pss2[:])

            tvsq = sb.tile([1, 1], F32, tag="tvsq")
            nc.vector.tensor_tensor(tvsq[:], tv_sb[:], tv_sb[:], OP.mult)
            dden = sb.tile([1, 1], F32, tag="dden")
            nc.vector.scalar_tensor_tensor(dden[:], tvsq[:], 1e-6, s2_sb[:],
                                           OP.mult, OP.add)
            rdden = sb.tile([1, 1], F32, tag="rdden")
            nc.vector.reciprocal(rdden[:], dden[:])
            eff_sb = sb.tile([1, 1], F32, tag="effsb")
            nc.vector.tensor_tensor(eff_sb[:], tvsq[:], rdden[:], OP.mult)

            # cdist tail: d2 = r_i + r_j - 2G
            rrow_ps = misc.tile([1, SN], F32, tag="m")
            nc.tensor.transpose(rrow_ps[:], r_sb[:], ident10[:])
            rrow = sb.tile([1, SN], F32, tag="rrow")
            nc.scalar.copy(rrow[:], rrow_ps[:])
            rB = misc.tile([SN, SN], F32, tag="m")
            nc.tensor.matmul(rB[:], ones1_10[:], rrow[:], start=True,
                             stop=True)
            d2 = sb.tile([SN, SN], F32, tag="d2")
            nc.vector.scalar_tensor_tensor(d2[:], g_sb[:], -2.0, rB[:],
                                           OP.mult, OP.add)
            nc.vector.tensor_scalar(d2[:], d2[:], r_sb[:], 0.0, OP.add,
                                    OP.max)
            dst = sb.tile([SN, SN], F32, tag="dst")
            nc.scalar.activation(dst[:], d2[:], ACT.Sqrt)
            dsum = sb.tile([SN, 1], F32, tag="dsum")
            nc.vector.tensor_reduce(dsum[:], dst[:], AX.X, OP.add)
            psD = misc.tile([1, 1], F32, tag="m")
            nc.tensor.matmul(psD[:], dsum[:], ones10[:], start=True, stop=True)
            avg_sb = sb.tile([1, 1], F32, tag="avgsb")
            nc.vector.tensor_scalar(avg_sb[:], psD[:],
                                    float(1.0 / (SN * (SN - 1) + 1e-6)), None,
                                    OP.mult)
            sqtv = sb.tile([1, 1], F32, tag="sqtv")
            nc.scalar.activation(sqtv[:], tv_sb[:], ACT.Sqrt)
            diff_sb = sb.tile([1, 1], F32, tag="diffsb")
            nc.vector.tensor_tensor(diff_sb[:], sqtv[:], avg_sb[:], OP.mult)
            tanhd = sb.tile([1, 1], F32, tag="tanhd")
            nc.scalar.activation(tanhd[:], diff_sb[:], ACT.Tanh)
            # load the Ln act table right after the last Tanh (input dep on
            # tanhd pins the scheduler) so no table switch hits the tail
            lnwarm = sb.tile([1, 1], F32, tag="lnwarm")
            nc.scalar.activation(lnwarm[:], tanhd[:], ACT.Ln)
            outrow = sb.tile([1, 9], F32, tag="outrow")
            nc.vector.tensor_copy(outrow[:, 1:2], diff_sb[:])
            nc.vector.tensor_copy(outrow[:, 2:3], eff_sb[:])
            nc.vector.tensor_copy(outrow[:, 3:4], tv_sb[:])

            # ---- stage A: stream HT; ht chunks stationary, masks moving ----
            # psAll[:, c*J+j] accumulates S.T[t, j] for t-chunk c: 128 t rows
            # on partitions, all 32 chunks x 8 series in half a PSUM bank.
            psAll = psA_pool.tile([128, NCH * J], F32, tag="sacc")
            for dk in range(NDC):
                if dk == 0:
                    htt = htt0
                elif dk == NDC - 1:
                    # quarters so the tail after the last byte only covers
                    # 8 matmuls
                    htt = htp.tile([128, TL], F32, tag="htt", name="htt")
                    qt = TL // 4
                    for h in range(4):
                        q = nc.sync if h % 2 == 0 else nc.gpsimd
                        q.dma_start(
                            out=htt[:, h * qt:(h + 1) * qt],
                            in_=ht[dk * 128:(dk + 1) * 128,
                                   h * qt:(h + 1) * qt])
                else:
                    htt = htp.tile([128, TL], F32, tag="htt", name="htt")
                    q = nc.sync if (dk % 2 == 0) else nc.gpsimd
                    q.dma_start(out=htt[:],
                                in_=ht[dk * 128:(dk + 1) * 128, :])
                for c in range(NCH):
                    # start zeroes the whole 2KB zero-region (bank), so only
                    # the very first matmul in the bank may carry start=True
                    nc.tensor.matmul(psAll[:, c * J:(c + 1) * J],
                                     htt[:, c * 128:(c + 1) * 128],
                                     m_sb[:, dk * J:(dk + 1) * J],
                                     start=(dk == 0 and c == 0),
                                     stop=(dk == NDC - 1 and c == NCH - 1),
                                     skip_group_check=True)

            # ---- stage B: raw min/max per series, scale, AllReduce(max) ----
            # mxmn cols 0:8 = max, cols 8:16 = -min (so one max-reduce after
            # transpose covers both); AR payload col1 carries invc (constant
            # across cores, so max is the identity on it)
            ps3 = psAll[:].rearrange("p (c j) -> p j c", j=J)
            mxmn = sb.tile([128, 2 * J], F32, tag="mxmn")
            nc.vector.tensor_reduce(mxmn[:, 0:J], ps3, AX.X, OP.max)
            nc.vector.tensor_reduce(mxmn[:, J:2 * J], ps3, AX.X, OP.min)
            nc.vector.tensor_scalar(mxmn[:, J:2 * J], mxmn[:, J:2 * J], -1.0,
                                    None, OP.mult)
            psT = misc.tile([2 * J, 128], F32, tag="m", name="psT")
            nc.tensor.transpose(psT[:], mxmn[:], ident128[:])
            minmax = sb.tile([2 * J, 2], F32, tag="minmax")
            nc.vector.tensor_copy(minmax[:, 1:2], invc_sb[:, 1:2])
            tmx = sb.tile([2 * J, 1], F32, tag="tmx")
            nc.vector.tensor_reduce(tmx[:], psT[:], AX.X, OP.max)
            nc.vector.tensor_scalar(minmax[:, 0:1], tmx[:],
                                    invc_sb[:, 0:1], None, OP.mult)
            cbA = dram.tile([2 * J, 2], F32, tag="cba")
            cbB = dram.tile([2 * J, 2], F32, tag="cbb")
            nc.sync.dma_start(out=cbA[:], in_=minmax[:])
            if sim1:
                nc.sync.dma_start(out=cbB[:], in_=cbA[:])
            else:
                nc.gpsimd.collective_compute("AllReduce", OP.max,
                                             replica_groups=rg,
                                             ins=[cbA.opt()],
                                             outs=[cbB.opt()])
            # read back replicated on every partition: grow[p, 2r+c]=cbB[r,c]
            grow = sb.tile([128, 4 * J], F32, tag="grow")
            nc.sync.dma_start(
                out=grow[:],
                in_=cbB[:].rearrange("r c -> (r c)")[None, :]
                .broadcast_to([128, 4 * J]))
            # row-wise: rng = max+(-min); s1 = 10/(rng+1e-6);
            # s1eff = s1*invc; b1 = (-min)*s1 - 0.5
            gmax = grow[:, 0:2 * J:2]
            gnmn = grow[:, 2 * J:4 * J:2]
            ginv = grow[:, 1:2 * J:2]
            rrow = sb.tile([128, J], F32, tag="rrow2")
            nc.vector.scalar_tensor_tensor(rrow[:], gmax, 1e-6, gnmn,
                                           OP.add, OP.add)
            nc.vector.reciprocal(rrow[:], rrow[:])
            s1eff = sb.tile([128, J], F32, tag="s1eff")
            nc.vector.tensor_tensor(s1eff[:], rrow[:], ginv, OP.mult)
            b1row = sb.tile([128, J], F32, tag="b1row")
            nc.vector.tensor_tensor(b1row[:], gnmn, rrow[:], OP.mult)
            nc.vector.tensor_scalar(b1row[:], b1row[:], 10.0, -0.5,
                                    OP.mult, OP.add)

            # ---- stage C: affine + int-cast + clamp + one-hot + joints ----
            binf = sb.tile([128, NCH * J], F32, tag="binf")
            b3 = binf[:].rearrange("p (c j) -> p c j", j=J)
            nc.vector.tensor_tensor(
                b3, psAll[:].rearrange("p (c j) -> p c j", j=J),
                s1eff[:, None, :].broadcast_to([128, NCH, J]), OP.mult)
            binint = sb.tile([128, NCH * J], I32, tag="binint")
            nc.vector.tensor_tensor(
                binint[:].rearrange("p (c j) -> p c j", j=J), b3,
                b1row[:, None, :].broadcast_to([128, NCH, J]),
                OP.add)
            nc.vector.tensor_scalar(binint[:], binint[:], 0, NB - 1, OP.max,
                                    OP.min)
            ohsb = sb.tile([128, NCH * J * NB], BF16, tag="ohsb")
            oh3 = ohsb[:].rearrange("p (c b) -> p c b", b=NB)
            for b in range(NB):
                eng = nc.vector if b < 7 else nc.gpsimd
                eng.tensor_scalar(oh3[:, :, b], binint[:], b, None,
                                  OP.is_equal)
            # joint histograms: all 4 pairs side by side in one PSUM bank
            psJt = psJ_pool.tile([NB, NPAIR * NB], F32, tag="pj")
            for c in range(NCH):
                for p in range(NPAIR):
                    xa = (c * J + 2 * p) * NB
                    ya = (c * J + 2 * p + 1) * NB
                    nc.tensor.matmul(psJt[:, p * NB:(p + 1) * NB],
                                     ohsb[:, xa:xa + NB],
                                     ohsb[:, ya:ya + NB],
                                     start=(c == 0 and p == 0),
                                     stop=(c == NCH - 1 and p == NPAIR - 1),
                                     skip_group_check=True)
            gjl = sb.tile([NB, NPAIR * NB], F32, tag="gjl")
            nc.vector.tensor_copy(gjl[:], psJt[:])
            cbj = dram.tile([NB, NPAIR * NB], F32, tag="cbj")
            cbj2 = dram.tile([NB, NPAIR * NB], F32, tag="cbj2")
            nc.sync.dma_start(out=cbj[:], in_=gjl[:])
            if sim1:
                nc.sync.dma_start(out=cbj2[:], in_=cbj[:])
            else:
                nc.gpsimd.collective_compute("AllReduce", OP.add,
                                             replica_groups=rg,
                                             ins=[cbj.opt()],
                                             outs=[cbj2.opt()])
            gj = sb.tile([NB, NPAIR * NB], F32, tag="gj")
            nc.sync.dma_start(out=gj[:], in_=cbj2[:])

            # ---- stage D: batched MI over the 4 pairs ----
            # mi_p = (1/T) sum_ij n_ij*(ln(n_ij+EPS_N)+LN_T-ln(r_i*c_j+EPS_RC))
            gj3 = gj[:].rearrange("a (p b) -> a p b", b=NB)
            r4 = sb.tile([NB, NPAIR], F32, tag="r4")
            nc.vector.tensor_reduce(r4[:], gj3, AX.X, OP.add)
            pscB = misc.tile([NB, NPAIR * NB], F32, tag="m", name="pscB")
            nc.tensor.matmul(pscB[:], ones10x10[:], gj[:], start=True,
                             stop=True)
            rc = sb.tile([NB, NPAIR * NB], F32, tag="rc")
            nc.vector.tensor_tensor(
                rc[:].rearrange("a (p b) -> a p b", b=NB),
                pscB[:].rearrange("a (p b) -> a p b", b=NB),
                r4[:, :, None].broadcast_to([NB, NPAIR, NB]), OP.mult)
            lnn = sb.tile([NB, NPAIR * NB], F32, tag="lnn")
            nc.scalar.activation(lnn[:], gj[:], ACT.Ln, bias=cepsn[:])
            lnrc = sb.tile([NB, NPAIR * NB], F32, tag="lnrc")
            nc.scalar.activation(lnrc[:], rc[:], ACT.Ln, bias=cepsrc[:])
            lterm = sb.tile([NB, NPAIR * NB], F32, tag="lterm")
            nc.vector.scalar_tensor_tensor(lterm[:], lnn[:], LN_T, lnrc[:],
                                           OP.add, OP.subtract)
            nc.vector.tensor_tensor(lterm[:], gj[:], lterm[:], OP.mult)
            rsum = sb.tile([NB, NPAIR], F32, tag="rsum")
            nc.vector.tensor_reduce(
                rsum[:], lterm[:].rearrange("a (p b) -> a p b", b=NB),
                AX.X, OP.add)
            psmi = misc.tile([1, NPAIR], F32, tag="m", name="psmi")
            nc.tensor.matmul(psmi[:], ones10[:], rsum[:], start=True,
                             stop=True)
            nc.vector.tensor_scalar(outrow[:, 5:9], psmi[:], INV_T, 0.0,
                                    OP.mult, OP.max)
            nc.vector.tensor_reduce(outrow[:, 4:5], outrow[:, 5:9], AX.X,
                                    OP.min)
            nc.vector.tensor_tensor(outrow[:, 0:1], outrow[:, 4:5], tanhd[:],
                                    OP.add)
            nc.sync.dma_start(out=out[:], in_=outrow[:])
            if debug:
                nc.sync.dma_start(out=dbg_st[:], in_=binf[:, 0:J])
                nc.sync.dma_start(out=dbg_gmm[:], in_=minmax[:])
                nc.sync.dma_start(out=dbg_s1b1[:, 0:J], in_=s1eff[0:1, :])
                nc.sync.dma_start(out=dbg_bin[:], in_=binint[:, 0:16])
                nc.sync.dma_start(out=dbg_gj[:], in_=gj[:])

    nc.compile()
    return nc


def _build_variant(name):
    return _build(variant=name)


def _get_nc(debug=False):
    key = ("ncd" if debug else "nc")
    if key not in _CACHE:
        _CACHE[key] = _build(debug)
    return _CACHE[key]


def kernel(state, state_memory, state_history, partitions, sample_idx,
           trace=False, debug=False):
    global LAST_RESULTS
    state = np.asarray(state, np.float32)
    state_memory = np.asarray(state_memory, np.float32)
    state_history = np.asarray(state_history, np.float32)
    partitions = np.asarray(partitions)
    sample_idx = np.asarray(sample_idx)

    mmat = np.empty((D, J), np.float32)
    invc8 = np.empty((J,), np.float32)
    pf = partitions.astype(np.float32)
    for p in range(NPAIR):
        mmat[:, 2 * p] = pf[p]
        mmat[:, 2 * p + 1] = np.float32(1.0) - pf[p]
        invc8[2 * p] = np.float32(1.0) / pf[p].sum(dtype=np.float32)
        invc8[2 * p + 1] = np.float32(1.0) / (np.float32(1.0) - pf[p]).sum(
            dtype=np.float32)
    invc = np.zeros((2 * J, 2), np.float32)
    invc[:, 0] = np.tile(invc8, 2)
    invc[0:J, 1] = np.float32(10.0) * invc8
    memory = np.concatenate([state, state_memory[state.shape[0]:]], axis=0)

    def _relayout(arrT, f):
        # [D, f] row-major -> [128, NDC*f]: row p holds chunks k at cols k*f
        return np.ascontiguousarray(
            arrT.reshape(NDC, 128, f).transpose(1, 0, 2).reshape(128, NDC * f))

    mmat = _relayout(mmat, J)
    memt = _relayout(np.ascontiguousarray(memory.T), MEM).astype(
        ml_dtypes.bfloat16)
    sampt = _relayout(np.ascontiguousarray(memory[sample_idx].T), SN).astype(
        ml_dtypes.bfloat16)

    in_maps = []
    for c in range(N_CORES):
        htc = np.ascontiguousarray(state_history[c * TL:(c + 1) * TL, :].T)
        in_maps.append({"ht": htc, "mmat": mmat, "invc": invc,
                        "memt": memt, "sampt": sampt})

    nc = _get_nc(debug)
    res = run_bass_kernel_spmd(nc, in_maps, list(range(N_CORES)),
                               trace=trace)
    LAST_RESULTS = res
    return np.asarray(res.results[0]["out"], np.float32)
